# revision 16
# baseline (speedup 1.0000x reference)
"""Trainium2 Bass kernel for nn_DeepSymmetricGCN1dBlock.

3-layer GCN block over a shared 2048-node graph, 32 graph copies (b=4, n=8),
channels 128->256->256->256, per-element branch + symmetric max-pooled branch,
training-mode BatchNorm, ReLU.

Strategy (v5)
-------------
Data-parallel over the 32 graph copies: core k holds copies of graph b=k//2,
n in [4*(k%2), 4*(k%2)+4).  The sparse GCN aggregation is a dense matmul
against the normalized adjacency A_hat [2048, 2048], kept RESIDENT in SBUF
in bf16 (8 MiB), streamed in N=512 moving chunks.  All matmul operands are
bf16 (PSUM accumulation stays fp32); BN statistics are fp32.

Layer 1 runs aggregation-first (agg = x^T A at Cin=128 width; x is uploaded
pre-transposed to node-major by the host), dqq-OUTER across the 4 element
instances so compute starts as soon as the first A chunk lands.  Layers 2-3
run W-first (h = x W, then y = h^T A); h psum is drained in [128,512] pairs
alternating ACT/DVE so drains keep up with the matmul stream.

Per layer: 4 element instances first, then element-stats AllReduce (hidden
under the pooled instance), then pooled-stats AllReduce.  During the pooled
AR flight the element BN affine is pre-applied in place
(t = a1*y1 + (b1+b2), DVE), so post-AR work is just
x' = relu(a2*y2 + t) per copy.  Pool-max AllReduce runs in bf16 (exact) and
lands during the next layer's element matmuls.  Layer 3 ships pre-BN y1/y2
plus stat sums; its pooled instance is pair-split by destination-node halves
(per-core Ash_pool input selects the half) and the host stitches + applies
the final BN affine + relu.
"""

import sys

if "/opt/trn_rl_repo" not in sys.path:
    sys.path.insert(0, "/opt/trn_rl_repo")

import numpy as np
import ml_dtypes

import concourse.bass as bass
import concourse.bacc as bacc
import concourse.mybir as mybir
import concourse.tile as tile
from concourse.bass_utils import run_bass_kernel_spmd

f32 = mybir.dt.float32
bf16 = mybir.dt.bfloat16
AF = mybir.ActivationFunctionType
OP = mybir.AluOpType
AX = mybir.AxisListType

B, N, L, E = 4, 8, 2048, 16384
CH = [128, 256, 256, 256]
EPS = 1e-5
NCORES = 8
GPC = 4            # graph copies per core
LT = L // 128      # 16 node tiles
DQQ = 4            # A streamed in DQQ chunks of DW destination columns
DW = L // DQQ      # 512
CNT_E = 32 * L     # element-branch BN count (all 32 copies)
CNT_P = 8 * L      # pooled-branch BN count L1/L2 (4 graphs, pair-redundant)
CNT_P3 = 4 * L     # pooled L3: pair-split, each node counted once

PAIRS = [[0, 1], [2, 3], [4, 5], [6, 7]]
ALL8 = [list(range(NCORES))]

import os
_PROFILE = False
_SIMULATE = False
_NO_CC = os.environ.get("K_NO_CC", "0") == "1"
_NFILL = int(os.environ.get("K_NFILL", "70"))
_CACHE = {}


def _cc(nc, kind, op, groups, bi, bo):
    """bi/bo are APs into DRAM bounce tiles."""
    if _NO_CC:
        nc.sync.dma_start(bo, bi)
    else:
        nc.gpsimd.collective_compute(kind, op, replica_groups=groups,
                                     ins=[bi.opt()], outs=[bo.opt()])


def _emit(tc, nc, io):
    sync, vec, act, te = nc.sync, nc.vector, nc.scalar, nc.tensor

    from contextlib import ExitStack

    ctx = ExitStack()
    with ctx:
        sb = ctx.enter_context(tc.tile_pool(name="sb", bufs=1))
        sb_slot = ctx.enter_context(tc.tile_pool(name="slots", bufs=GPC + 1))
        sb_y1 = ctx.enter_context(tc.tile_pool(name="y1", bufs=GPC + 2))
        sb_h = ctx.enter_context(tc.tile_pool(name="h", bufs=2))
        sb_agg = ctx.enter_context(tc.tile_pool(name="agg", bufs=3))
        sb_w = ctx.enter_context(tc.tile_pool(name="w", bufs=6))
        sb_small = ctx.enter_context(tc.tile_pool(name="small", bufs=28))
        ps_a = ctx.enter_context(tc.tile_pool(name="psa", bufs=2, space="PSUM"))
        ps_w = ctx.enter_context(tc.tile_pool(name="psw", bufs=2, space="PSUM"))
        ps_h = ctx.enter_context(tc.tile_pool(name="psh", bufs=2, space="PSUM"))
        ps_f = ctx.enter_context(tc.tile_pool(name="psf", bufs=1, space="PSUM"))
        dram = ctx.enter_context(tc.tile_pool(name="dram", bufs=1, space="DRAM"))

        # ---- persistent SBUF tiles -------------------------------------
        Asb = sb.tile([128, DQQ * LT * DW], bf16, tag="Asb")
        Apsb = sb.tile([128, 2 * LT * DW], bf16, tag="Apsb")  # L3 pool half
        slots = [sb_slot.tile([128, LT * 128], bf16, tag="slot", name=f"slot{i}")
                 for i in range(GPC)]
        pool_nm = sb_slot.tile([128, LT * 128], bf16, tag="slot", name="pool_nm")
        y1s = [sb_y1.tile([128, 2 * L], bf16, tag="y1", name=f"y1_{i}")
               for i in range(GPC + 1)]
        poolb = sb_y1.tile([128, 2 * L], bf16, tag="y1", name="poolb")
        mxsb = sb.tile([128, 2 * L], bf16, tag="mxsb")
        sqscr = sb.tile([128, L], bf16, tag="sqscr")
        wsb = [sb_w.tile([128, 512], bf16, tag="w", name=f"w{i}") for i in range(6)]
        gbt = sb.tile([128, 24], f32, tag="gbt")
        s1e = sb.tile([128, 2 * GPC * DQQ], f32, tag="s1e")   # [cot][g][dqq]
        s1p = sb.tile([128, 2 * DQQ], f32, tag="s1p")         # [cot][dqq]
        s2e = sb.tile([128, 2 * GPC * DQQ], f32, tag="s2e")   # [cot][g][dqq]
        s2p = sb.tile([128, 2 * DQQ], f32, tag="s2p")         # [cot][dqq]
        packe = sb.tile([128, 4], f32, tag="packe")
        packp = sb.tile([128, 4], f32, tag="packp")
        globe = sb.tile([128, 4], f32, tag="globe")
        globp = sb.tile([128, 4], f32, tag="globp")

        fpsum = ps_f.tile([128, DW], f32, tag="fpsum")

        # ---- DRAM bounce tiles for collectives -------------------------
        mx_in = dram.tile([128, 2 * L], bf16, tag="mxi")
        mx_out = dram.tile([128, 2 * L], bf16, tag="mxo")
        ste_in = dram.tile([128, 4], f32, tag="stei")
        ste_outs = [dram.tile([128, 4], f32, tag="steo", name=f"ste_out{i}",
                              addr_space="Shared") for i in range(2)]
        warm_in = dram.tile([128, 1], f32, tag="warmi")
        warm_out = dram.tile([128, 1], f32, tag="warmo", addr_space="Shared")
        warm2_in = dram.tile([128, 1], f32, tag="warm2i")
        warm2_out = dram.tile([128, 1], f32, tag="warm2o", addr_space="Shared")
        stp_in = dram.tile([128, 4], f32, tag="stpi")
        stp_outs = [dram.tile([128, 4], f32, tag="stpo", name=f"stp_out{i}",
                              addr_space="Shared") for i in range(2)]

        xsh_d, px_d, ash_d, ashp_d, w_d, gb_d, out_d, out2_d, outst_d = (
            io["xsh"], io["pxsh"], io["Ash"], io["AshP"], io["Wmats"],
            io["gbs"], io["out"], io["out2"], io["outst"])

        # small affine tiles
        eps_t = sb_small.tile([128, 1], f32, tag="sm", name="eps")
        vec.memset(eps_t[:], EPS)
        t0 = sb_small.tile([128, 2], f32, tag="sm", name="t0")
        a1 = sb_small.tile([128, 2], f32, tag="sm", name="a1")
        b1 = sb_small.tile([128, 2], f32, tag="sm", name="b1")
        a2 = sb_small.tile([128, 2], f32, tag="sm", name="a2")
        a2h = sb_small.tile([128, 2], bf16, tag="sm", name="a2h")
        bs = sb_small.tile([128, 2], f32, tag="bs", name="bs")
        me = sb_small.tile([128, 2], f32, tag="sm", name="me")
        ve = sb_small.tile([128, 2], f32, tag="sm", name="ve")


        # ---- input loads: g0's first A-block needs slot0 + Ash[0] only
        sync.dma_start(slots[0][:], xsh_d[0, :, :])
        sync.dma_start(Asb[:, 0:LT * DW], ash_d[:, 0, :])
        sync.dma_start(wsb[0][:], w_d[0, :, :])
        # warm-up collective #1: absorb the 8-core rendezvous skew early,
        # while the tensor engine chews on layer-1 matmuls.
        sync.dma_start(warm_in[:], eps_t[:])
        _cc(nc, "AllReduce", OP.add, ALL8, warm_in[:], warm_out[:])
        for g in range(1, GPC):
            sync.dma_start(slots[g][:], xsh_d[g, :, :])
        for dqq in range(1, DQQ):
            sync.dma_start(Asb[:, dqq * LT * DW:(dqq + 1) * LT * DW],
                           ash_d[:, dqq, :])
        sync.dma_start(pool_nm[:], px_d[:, :])
        for i in [3, 1, 4, 2, 5]:
            sync.dma_start(wsb[i][:], w_d[i, :, :])
        sync.dma_start(gbt[:], gb_d[:, :])
        sync.dma_start(Apsb[:, 0:LT * DW], ashp_d[:, 0, :])
        sync.dma_start(Apsb[:, LT * DW:2 * LT * DW], ashp_d[:, 1, :])

        def affine(a_t, b_t, s1_ap, s2_ap, inv_cnt, gslc, beslc):
            # a = g * rsqrt(var+eps); b = be - a*mean
            vec.tensor_scalar(me[:], s1_ap, inv_cnt, None, OP.mult)
            vec.tensor_scalar(ve[:], s2_ap, inv_cnt, None, OP.mult)
            vec.tensor_tensor(t0[:], me[:], me[:], OP.mult)
            vec.tensor_tensor(ve[:], ve[:], t0[:], OP.subtract)
            act.activation(t0[:], ve[:], AF.Sqrt, bias=eps_t[:])
            vec.reciprocal(t0[:], t0[:])
            vec.tensor_tensor(a_t[:], gslc, t0[:], OP.mult)
            vec.tensor_tensor(t0[:], a_t[:], me[:], OP.mult)
            vec.tensor_tensor(b_t[:], beslc, t0[:], OP.subtract)

        def sq_chunk(y_ap, s2_slot):
            """Sum of squares of one [128, DW] drained chunk on DVE."""
            vec.scalar_tensor_tensor(sqscr[:, 0:DW], y_ap, 1.0, y_ap,
                                     OP.mult, OP.mult, accum_out=s2_slot)

        def emit_A_block(src, dqq, drain_dve):
            """L1-style: agg[:, :] = x_chunk^T A[:, dqq block]; one psum."""
            pa = ps_a.tile([128, DW], f32, tag="psa")
            for st in range(LT):
                te.matmul(pa[:], src[:, st * 128:(st + 1) * 128],
                          Asb[:, (dqq * LT + st) * DW:(dqq * LT + st + 1) * DW],
                          start=(st == 0), stop=(st == LT - 1))
            agg = sb_agg.tile([128, DW], bf16, tag="agg")
            if drain_dve:
                vec.tensor_copy(agg[:], pa[:])
            else:
                act.activation(agg[:], pa[:], AF.Copy)
            return agg

        def emit_W_block(li, g, dqq, agg):
            """Project agg (Cin wide) to the two cot halves of y."""
            we = wsb[li] if g < GPC else wsb[3 + li]
            dsty = y1s[g] if g < GPC else y1s[GPC]
            s1 = s1e if g < GPC else s1p
            s2 = s2e if g < GPC else s2p
            for cot in range(2):
                pw = ps_w.tile([128, DW], f32, tag="psw")
                te.matmul(pw[:], we[:, cot * 128:(cot + 1) * 128], agg[:],
                          start=True, stop=True)
                idx = (cot * GPC + g) * DQQ + dqq if g < GPC \
                    else cot * DQQ + dqq
                yap = dsty[:, cot * L + dqq * DW:cot * L + (dqq + 1) * DW]
                act.activation(yap, pw[:], AF.Copy,
                               accum_out=s1[:, idx:idx + 1])
                sq_chunk(yap, s2[:, idx:idx + 1])

        def emit_h(li, g, src):
            """h = src W for L2/L3; paired [128,512] psum, alt ACT/DVE drain."""
            we = wsb[li] if g < GPC else wsb[3 + li]
            h = sb_h.tile([128, LT * 256], bf16, tag="h")
            for sp in range(LT // 2):
                ph = ps_h.tile([128, 512], f32, tag="psh")
                for sub in range(2):
                    st = sp * 2 + sub
                    for ct in range(2):
                        te.matmul(ph[:, sub * 256:(sub + 1) * 256],
                                  src[:, ct * L + st * 128:
                                      ct * L + st * 128 + 128],
                                  we[:, ct * 256:(ct + 1) * 256],
                                  start=(ct == 0), stop=(ct == 1))
                if sp % 2 == 0:
                    act.activation(h[:, sp * 512:(sp + 1) * 512], ph[:],
                                   AF.Copy)
                else:
                    vec.tensor_copy(h[:, sp * 512:(sp + 1) * 512], ph[:])
            return h

        def emit_A(li, g, h, nq=DQQ, asrc=None, out_dram=None):
            """y[cot, dqq] = h^T A for L2/L3 (dqq-major, st accumulation)."""
            if asrc is None:
                asrc = Asb
            dsty = y1s[g] if g < GPC else y1s[GPC]
            s1 = s1e if g < GPC else s1p
            s2 = s2e if g < GPC else s2p
            for cot in range(2):
                for dqq in range(nq):
                    pw = ps_w.tile([128, DW], f32, tag="psw")
                    for st in range(LT):
                        te.matmul(pw[:],
                                  h[:, st * 256 + cot * 128:
                                    st * 256 + cot * 128 + 128],
                                  asrc[:, (dqq * LT + st) * DW:
                                       (dqq * LT + st + 1) * DW],
                                  start=(st == 0), stop=(st == LT - 1))
                    if g < GPC:
                        idx = (cot * GPC + g) * DQQ + dqq
                        col = cot * L + dqq * DW
                    else:
                        idx = cot * nq + dqq
                        col = cot * nq * DW + dqq * DW
                    yap = dsty[:, col:col + DW]
                    act.activation(yap, pw[:], AF.Copy,
                                   accum_out=s1[:, idx:idx + 1])
                    sq_chunk(yap, s2[:, idx:idx + 1])
                    if out_dram is not None:
                        sync.dma_start(out_dram[:, col:col + DW], yap)

        def pack_launch_e(li):
            for cot in range(2):
                vec.reduce_sum(packe[:, cot:cot + 1],
                               s1e[:, cot * GPC * DQQ:(cot + 1) * GPC * DQQ],
                               axis=AX.X)
                vec.reduce_sum(packe[:, 2 + cot:3 + cot],
                               s2e[:, cot * GPC * DQQ:(cot + 1) * GPC * DQQ],
                               axis=AX.X)
            sync.dma_start(ste_in[:], packe[:])
            _cc(nc, "AllReduce", OP.add, ALL8, ste_in[:], ste_outs[li][:])
            sync.dma_start(globe[:], ste_outs[li][:])

        def pack_launch_p(li, nq=DQQ):
            for cot in range(2):
                vec.reduce_sum(packp[:, cot:cot + 1],
                               s1p[:, cot * nq:(cot + 1) * nq], axis=AX.X)
                vec.reduce_sum(packp[:, 2 + cot:3 + cot],
                               s2p[:, cot * nq:(cot + 1) * nq], axis=AX.X)
            sync.dma_start(stp_in[:], packp[:])
            _cc(nc, "AllReduce", OP.add, ALL8, stp_in[:], stp_outs[li][:])
            sync.dma_start(globp[:], stp_outs[li][:])

        def prescale(li):
            # during AR_p flight: affine_e then t = a1*y1 + (b1+b2)... but
            # b2 needs pooled stats; use t = a1*y1 + b1 and fold b2 into the
            # pooled-side bias after AR_p.
            affine(a1, b1, globe[:, 0:2], globe[:, 2:4], 1.0 / CNT_E,
                   gbt[:, 4 * li:4 * li + 2], gbt[:, 4 * li + 2:4 * li + 4])
            for g in range(GPC):
                for cot in range(2):
                    vec.tensor_scalar(
                        y1s[g][:, cot * L:(cot + 1) * L],
                        y1s[g][:, cot * L:(cot + 1) * L],
                        a1[:, cot:cot + 1], b1[:, cot:cot + 1],
                        OP.mult, OP.add)

        def y2v_scale(nchunks=2):
            # v = a2*y2 + b2 into poolb (free at the boundary); phase3 then
            # only needs an add + relu per copy.
            cl = L // nchunks
            for ch in range(nchunks):
                for cot in range(2):
                    sl = slice(cot * L + ch * cl, cot * L + (ch + 1) * cl)
                    vec.tensor_scalar(poolb[:, sl], y1s[GPC][:, sl],
                                      a2[:, cot:cot + 1], bs[:, cot:cot + 1],
                                      OP.mult, OP.add)

        def phase3_g(li, g, nchunks=1):
            # x'_g = relu(t_g + v);  t = a1*y1+b1 (prescaled), v in poolb
            cl = L // nchunks
            for ch in range(nchunks):
                for cot in range(2):
                    sl = slice(cot * L + ch * cl, cot * L + (ch + 1) * cl)
                    vec.tensor_tensor(y1s[g][:, sl], poolb[:, sl],
                                      y1s[g][:, sl], OP.add)
                    act.activation(y1s[g][:, sl], y1s[g][:, sl], AF.Relu)

        def fillers(n):
            # junk matmuls that keep the PE HAM clock-gate warm across a
            # collective wait; never read back.  The rhs reads the pooled
            # instance's last-drained chunk so the scheduler cannot hoist
            # the batch away from the layer boundary it must cover.
            for _ in range(n):
                te.matmul(fpsum[:], wsb[0][:, 0:128],
                          y1s[GPC][:, 2 * L - DW:2 * L],
                          start=True, stop=True)

        # ================= LAYER 1 (agg-first, dqq-outer) ================
        pendW = None           # (g, dqq, agg) carried one block behind
        for dqq in range(DQQ):
            for g in range(GPC):
                agg = emit_A_block(slots[g], dqq, drain_dve=(g % 2 == 1))
                if pendW is not None:
                    emit_W_block(0, pendW[0], pendW[1], pendW[2])
                pendW = (g, dqq, agg)
        emit_W_block(0, pendW[0], pendW[1], pendW[2])
        pack_launch_e(0)

        # pooled L1 instance (hides AR_e)
        pendW = None
        for dqq in range(DQQ):
            agg = emit_A_block(pool_nm, dqq, drain_dve=(dqq % 2 == 1))
            if pendW is not None:
                emit_W_block(0, GPC, pendW[1], pendW[2])
            pendW = (GPC, dqq, agg)
        emit_W_block(0, GPC, pendW[1], pendW[2])
        pack_launch_p(0)
        fillers(_NFILL)
        prescale(0)

        # ================= LAYERS 2..3 ===================================
        for li in (1, 2):
            last = (li == 2)
            # boundary: affine_p, then per-copy phase3 feeding this layer
            affine(a2, bs, globp[:, 0:2], globp[:, 2:4], 1.0 / CNT_P,
                   gbt[:, 12 + 4 * (li - 1):14 + 4 * (li - 1)],
                   gbt[:, 14 + 4 * (li - 1):16 + 4 * (li - 1)])
            y2v_scale()

            hs = [None] * (GPC + 1)
            phase3_g(li, 0, nchunks=2)
            hs[0] = emit_h(li, 0, y1s[0])
            if not last:
                for g in range(1, GPC):
                    if g == 2:
                        # warm-up collective #2: re-sync cores mid-layer so
                        # the boundary AllReduces see minimal arrival skew
                        # (anchored on this layer's first accum slot).
                        sync.dma_start(warm2_in[:], s1e[:, 0:1])
                        _cc(nc, "AllReduce", OP.add, ALL8,
                            warm2_in[:], warm2_out[:])
                    phase3_g(li, g)
                    if g == 1:
                        vec.tensor_tensor(mxsb[:], y1s[0][:], y1s[1][:],
                                          OP.max)
                    else:
                        vec.tensor_tensor(mxsb[:], mxsb[:], y1s[g][:], OP.max)
                    emit_A(li, g - 1, hs[g - 1])
                    hs[g] = emit_h(li, g, y1s[g])
                    if g == GPC - 1:
                        # pool-max exchange feeding THIS layer's pooled inst
                        sync.dma_start(mx_in[:], mxsb[:])
                        _cc(nc, "AllReduce", OP.max, PAIRS,
                            mx_in[:], mx_out[:])
                        sync.dma_start(poolb[:], mx_out[:])
                emit_A(li, GPC - 1, hs[GPC - 1])
                pack_launch_e(1)
                # pooled instance (hides AR_e)
                hs[GPC] = emit_h(li, GPC, poolb)
                emit_A(li, GPC, hs[GPC])
                pack_launch_p(1)
                fillers(_NFILL)
                prescale(li)
            else:
                # L3 tensor order: h0 A0 h1 A1 h2 A2 hp Ap h3 A3; pre-BN
                # y1/y2 halves + stat sums stream out, host finishes BN+relu.
                phase3_g(li, 1)
                vec.tensor_tensor(mxsb[:], y1s[0][:], y1s[1][:], OP.max)
                emit_A(li, 0, hs[0], out_dram=out_d[0, :, :])
                hs[1] = emit_h(li, 1, y1s[1])
                phase3_g(li, 2)
                vec.tensor_tensor(mxsb[:], mxsb[:], y1s[2][:], OP.max)
                emit_A(li, 1, hs[1], out_dram=out_d[1, :, :])
                hs[2] = emit_h(li, 2, y1s[2])
                phase3_g(li, 3)
                vec.tensor_tensor(mxsb[:], mxsb[:], y1s[3][:], OP.max)
                sync.dma_start(mx_in[:], mxsb[:])
                _cc(nc, "AllReduce", OP.max, PAIRS, mx_in[:], mx_out[:])
                sync.dma_start(poolb[:], mx_out[:])
                emit_A(li, 2, hs[2], out_dram=out_d[2, :, :])
                hs[GPC] = emit_h(li, GPC, poolb)
                emit_A(li, GPC, hs[GPC], nq=2, asrc=Apsb,
                       out_dram=out2_d)
                for cot in range(2):
                    vec.reduce_sum(packp[:, cot:cot + 1],
                                   s1p[:, cot * 2:(cot + 1) * 2], axis=AX.X)
                    vec.reduce_sum(packp[:, 2 + cot:3 + cot],
                                   s2p[:, cot * 2:(cot + 1) * 2], axis=AX.X)
                sync.dma_start(outst_d[:, 4:8], packp[:])
                hs[3] = emit_h(li, 3, y1s[3])
                emit_A(li, 3, hs[3], out_dram=out_d[3, :, :])
                for cot in range(2):
                    vec.reduce_sum(packe[:, cot:cot + 1],
                                   s1e[:, cot * GPC * DQQ:
                                       (cot + 1) * GPC * DQQ], axis=AX.X)
                    vec.reduce_sum(packe[:, 2 + cot:3 + cot],
                                   s2e[:, cot * GPC * DQQ:
                                       (cot + 1) * GPC * DQQ], axis=AX.X)
                sync.dma_start(outst_d[:, 0:4], packe[:])


def _build():
    key = ("nc", _NO_CC)
    if key in _CACHE:
        return _CACHE[key]
    nc = bacc.Bacc("TRN2", target_bir_lowering=False, debug=False,
                   num_devices=NCORES)
    io = {
        "xsh": nc.dram_tensor("xsh", [GPC, 128, LT * 128], bf16,
                              kind="ExternalInput"),
        "pxsh": nc.dram_tensor("pxsh", [128, LT * 128], bf16,
                               kind="ExternalInput"),
        "Ash": nc.dram_tensor("Ash", [128, DQQ, LT * DW], bf16,
                              kind="ExternalInput"),
        "AshP": nc.dram_tensor("AshP", [128, 2, LT * DW], bf16,
                               kind="ExternalInput"),
        "Wmats": nc.dram_tensor("Wmats", [6, 128, 512], bf16,
                                kind="ExternalInput"),
        "gbs": nc.dram_tensor("gbs", [128, 24], f32, kind="ExternalInput"),
        "out": nc.dram_tensor("out", [GPC, 128, 2 * L], bf16,
                              kind="ExternalOutput"),
        "out2": nc.dram_tensor("out2", [128, 4 * DW], bf16,
                               kind="ExternalOutput"),
        "outst": nc.dram_tensor("outst", [128, 8], f32,
                                kind="ExternalOutput"),
    }
    with tile.TileContext(nc) as tc:
        _emit(tc, nc, io)
    nc.compile()
    _CACHE[key] = nc
    return nc


def _bf16(a):
    return np.asarray(a, np.float32).astype(ml_dtypes.bfloat16)


def _host_prep(edge_index, Ws, gs, bes):
    """Build the device-layout arrays on host."""
    src = np.asarray(edge_index[0], dtype=np.int64)
    dst = np.asarray(edge_index[1], dtype=np.int64)
    deg = np.zeros(L, np.float32)
    np.add.at(deg, dst, np.float32(1.0))
    deg += np.float32(2.0)
    dis = (1.0 / np.sqrt(deg.astype(np.float64))).astype(np.float32)
    A = np.zeros((L, L), np.float32)
    np.add.at(A, (src, dst), dis[src] * dis[dst])
    A[np.arange(L), np.arange(L)] += np.float32(2.0) * dis * dis
    ash = _bf16(np.ascontiguousarray(
        A.reshape(LT, 128, DQQ, DW).transpose(1, 2, 0, 3).reshape(128, DQQ, LT * DW)))

    wm = np.zeros((6, 128, 512), np.float32)
    for i, W in enumerate(Ws):
        cin = W.shape[0]
        wm[i, :, : (cin // 128) * 256] = np.ascontiguousarray(
            W.reshape(cin // 128, 128, 256).transpose(1, 0, 2).reshape(128, -1))
    wm = _bf16(wm)

    gb = np.zeros((128, 24), np.float32)
    vecs = [gs[0], bes[0], gs[1], bes[1], gs[2], bes[2],
            gs[3], bes[3], gs[4], bes[4], gs[5], bes[5]]
    for v, w in enumerate(vecs):
        gb[:, v * 2 + 0] = w[0:128]
        gb[:, v * 2 + 1] = w[128:256]
    return ash, wm, gb


def kernel(x, edge_index, W1, b1, W2, b2, W3, b3, W1s, b1s, W2s, b2s, W3s, b3s,
           g1, be1, g2, be2, g3, be3, g1s, be1s, g2s, be2s, g3s, be3s):
    x = np.asarray(x, np.float32)
    ash, wm, gb = _host_prep(
        np.asarray(edge_index),
        [np.asarray(W1, np.float32), np.asarray(W2, np.float32),
         np.asarray(W3, np.float32), np.asarray(W1s, np.float32),
         np.asarray(W2s, np.float32), np.asarray(W3s, np.float32)],
        [np.asarray(g1, np.float32), np.asarray(g2, np.float32),
         np.asarray(g3, np.float32), np.asarray(g1s, np.float32),
         np.asarray(g2s, np.float32), np.asarray(g3s, np.float32)],
        [np.asarray(be1, np.float32), np.asarray(be2, np.float32),
         np.asarray(be3, np.float32), np.asarray(be1s, np.float32),
         np.asarray(be2s, np.float32), np.asarray(be3s, np.float32)])

    # core k: graph b=k//2, copies n in [4*(k%2), 4*(k%2)+4)
    # upload x node-major: slot[p, st*128 + c] = x[st*128+p, c]
    xr = x.reshape(NCORES, GPC, CH[0], L)
    xnm = _bf16(np.ascontiguousarray(
        xr.reshape(NCORES, GPC, 128, LT, 128).transpose(0, 1, 4, 3, 2)
          .reshape(NCORES, GPC, 128, LT * 128)))
    # pooled-branch input: max over the 8 copies of each graph, node-major
    xp = x.reshape(B, N, CH[0], L).max(axis=1)  # [B, 128, L]
    xpnm = _bf16(np.ascontiguousarray(
        xp.reshape(B, 128, LT, 128).transpose(0, 3, 2, 1)
          .reshape(B, 128, LT * 128)))
    in_maps = []
    for k in range(NCORES):
        par = k % 2
        in_maps.append({
            "xsh": xnm[k], "pxsh": xpnm[k // 2],
            "Ash": ash, "AshP": np.ascontiguousarray(ash[:, 2 * par:2 * par + 2, :]),
            "Wmats": wm, "gbs": gb,
        })

    nc = _build()

    if _SIMULATE:
        from concourse.bass_interp import MultiCoreSim
        sim = MultiCoreSim(nc, NCORES)
        for k in range(NCORES):
            for nm, arr in in_maps[k].items():
                sim.cores[k].tensor(nm)[:] = arr
        sim.simulate(check_with_hw=False)
        outs = [np.array(sim.cores[k].mem_tensor("out")) for k in range(NCORES)]
        outs2 = [np.array(sim.cores[k].mem_tensor("out2")) for k in range(NCORES)]
        outsst = [np.array(sim.cores[k].mem_tensor("outst")) for k in range(NCORES)]
    else:
        res = run_bass_kernel_spmd(nc, in_maps, core_ids=list(range(NCORES)),
                                   trace=_PROFILE)
        if _PROFILE:
            _CACHE["last_result"] = res
        outs = [np.asarray(res.results[k]["out"]) for k in range(NCORES)]
        outs2 = [np.asarray(res.results[k]["out2"]) for k in range(NCORES)]
        outsst = [np.asarray(res.results[k]["outst"]) for k in range(NCORES)]

    # ---- host-side final layer: BN affine + pooled add + relu ----------
    # buf [GPC, 128, 2*L] bf16: y[g, cot*128+p, n] = buf[g, p, cot*L+n]
    y1 = np.stack([o.astype(np.float32) for o in outs])        # [8,G,128,2L]
    y1 = (y1.reshape(NCORES, GPC, 128, 2, L).transpose(0, 1, 3, 2, 4)
            .reshape(NCORES, GPC, 256, L))
    # out2 halves: core 2b+par holds dest-node cols (2*par+d2)*512+j
    y2 = np.empty((B, 256, L), np.float32)
    for b in range(B):
        for par in range(2):
            h = outs2[2 * b + par].astype(np.float32)  # [128, 4*DW]
            h = h.reshape(128, 2, 2, DW)               # [p, cot, d2, j]
            for cot in range(2):
                for d2 in range(2):
                    nd = (2 * par + d2) * DW
                    y2[b, cot * 128:(cot + 1) * 128, nd:nd + DW] = \
                        h[:, cot, d2, :]
    st = np.sum(np.stack([o.astype(np.float64) for o in outsst]), axis=0)

    def bn_affine(s1, s2, cnt, g, be):
        m = s1 / cnt
        v = s2 / cnt - m * m
        a = np.asarray(g, np.float64) / np.sqrt(v + EPS)
        return (a.astype(np.float32),
                (np.asarray(be, np.float64) - a * m).astype(np.float32))

    a1, b1 = bn_affine(st[:, 0:2].T.reshape(256), st[:, 2:4].T.reshape(256),
                       CNT_E, g3, be3)
    a2, b2 = bn_affine(st[:, 4:6].T.reshape(256), st[:, 6:8].T.reshape(256),
                       CNT_P3, g3s, be3s)
    bsum = (b1 + b2)[None, :, None]
    out = np.empty((NCORES * GPC, 256, L), np.float32)
    for k in range(NCORES):
        out[k * GPC:(k + 1) * GPC] = (a1[None, :, None] * y1[k]
                                      + a2[None, :, None] * y2[k // 2] + bsum)
    np.maximum(out, 0.0, out=out)
    return out


# revision 17
# speedup vs baseline: 1.0174x; 1.0174x over previous
"""Trainium2 Bass kernel for nn_DeepSymmetricGCN1dBlock.

3-layer GCN block over a shared 2048-node graph, 32 graph copies (b=4, n=8),
channels 128->256->256->256, per-element branch + symmetric max-pooled branch,
training-mode BatchNorm, ReLU.

Strategy (v5)
-------------
Data-parallel over the 32 graph copies: core k holds copies of graph b=k//2,
n in [4*(k%2), 4*(k%2)+4).  The sparse GCN aggregation is a dense matmul
against the normalized adjacency A_hat [2048, 2048], kept RESIDENT in SBUF
in bf16 (8 MiB), streamed in N=512 moving chunks.  All matmul operands are
bf16 (PSUM accumulation stays fp32); BN statistics are fp32.

Layer 1 runs aggregation-first (agg = x^T A at Cin=128 width; x is uploaded
pre-transposed to node-major by the host), dqq-OUTER across the 4 element
instances so compute starts as soon as the first A chunk lands.  Layers 2-3
run W-first (h = x W, then y = h^T A); h psum is drained in [128,512] pairs
alternating ACT/DVE so drains keep up with the matmul stream.

Per layer: 4 element instances first, then element-stats AllReduce (hidden
under the pooled instance), then pooled-stats AllReduce.  During the pooled
AR flight the element BN affine is pre-applied in place
(t = a1*y1 + (b1+b2), DVE), so post-AR work is just
x' = relu(a2*y2 + t) per copy.  Pool-max AllReduce runs in bf16 (exact) and
lands during the next layer's element matmuls.  Layer 3 ships pre-BN y1/y2
plus stat sums; its pooled instance is pair-split by destination-node halves
(per-core Ash_pool input selects the half) and the host stitches + applies
the final BN affine + relu.
"""

import sys

if "/opt/trn_rl_repo" not in sys.path:
    sys.path.insert(0, "/opt/trn_rl_repo")

import numpy as np
import ml_dtypes

import concourse.bass as bass
import concourse.bacc as bacc
import concourse.mybir as mybir
import concourse.tile as tile
from concourse.bass_utils import run_bass_kernel_spmd

f32 = mybir.dt.float32
bf16 = mybir.dt.bfloat16
AF = mybir.ActivationFunctionType
OP = mybir.AluOpType
AX = mybir.AxisListType

B, N, L, E = 4, 8, 2048, 16384
CH = [128, 256, 256, 256]
EPS = 1e-5
NCORES = 8
GPC = 4            # graph copies per core
LT = L // 128      # 16 node tiles
DQQ = 4            # A streamed in DQQ chunks of DW destination columns
DW = L // DQQ      # 512
CNT_E = 32 * L     # element-branch BN count (all 32 copies)
CNT_P = 8 * L      # pooled-branch BN count L1/L2 (4 graphs, pair-redundant)
CNT_P3 = 4 * L     # pooled L3: pair-split, each node counted once

PAIRS = [[0, 1], [2, 3], [4, 5], [6, 7]]
ALL8 = [list(range(NCORES))]

import os
_PROFILE = False
_SIMULATE = False
_NO_CC = os.environ.get("K_NO_CC", "0") == "1"
_NFILL = int(os.environ.get("K_NFILL", "70"))
_CACHE = {}


def _cc(nc, kind, op, groups, bi, bo):
    """bi/bo are APs into DRAM bounce tiles."""
    if _NO_CC:
        nc.sync.dma_start(bo, bi)
    else:
        nc.gpsimd.collective_compute(kind, op, replica_groups=groups,
                                     ins=[bi.opt()], outs=[bo.opt()])


def _emit(tc, nc, io):
    sync, vec, act, te = nc.sync, nc.vector, nc.scalar, nc.tensor

    from contextlib import ExitStack

    ctx = ExitStack()
    with ctx:
        sb = ctx.enter_context(tc.tile_pool(name="sb", bufs=1))
        sb_slot = ctx.enter_context(tc.tile_pool(name="slots", bufs=GPC + 1))
        sb_y1 = ctx.enter_context(tc.tile_pool(name="y1", bufs=GPC + 2))
        sb_h = ctx.enter_context(tc.tile_pool(name="h", bufs=2))
        sb_agg = ctx.enter_context(tc.tile_pool(name="agg", bufs=3))
        sb_w = ctx.enter_context(tc.tile_pool(name="w", bufs=6))
        sb_small = ctx.enter_context(tc.tile_pool(name="small", bufs=28))
        ps_a = ctx.enter_context(tc.tile_pool(name="psa", bufs=2, space="PSUM"))
        ps_w = ctx.enter_context(tc.tile_pool(name="psw", bufs=2, space="PSUM"))
        ps_h = ctx.enter_context(tc.tile_pool(name="psh", bufs=2, space="PSUM"))
        ps_f = ctx.enter_context(tc.tile_pool(name="psf", bufs=1, space="PSUM"))
        dram = ctx.enter_context(tc.tile_pool(name="dram", bufs=1, space="DRAM"))

        # ---- persistent SBUF tiles -------------------------------------
        Asb = sb.tile([128, DQQ * LT * DW], bf16, tag="Asb")
        Apsb = sb.tile([128, 2 * LT * DW], bf16, tag="Apsb")  # L3 pool half
        slots = [sb_slot.tile([128, LT * 128], bf16, tag="slot", name=f"slot{i}")
                 for i in range(GPC)]
        pool_nm = sb_slot.tile([128, LT * 128], bf16, tag="slot", name="pool_nm")
        y1s = [sb_y1.tile([128, 2 * L], bf16, tag="y1", name=f"y1_{i}")
               for i in range(GPC + 1)]
        poolb = sb_y1.tile([128, 2 * L], bf16, tag="y1", name="poolb")
        mxsb = sb.tile([128, 2 * L], bf16, tag="mxsb")
        sqscr = sb.tile([128, L], bf16, tag="sqscr")
        wsb = [sb_w.tile([128, 512], bf16, tag="w", name=f"w{i}") for i in range(6)]
        gbt = sb.tile([128, 24], f32, tag="gbt")
        s1e = sb.tile([128, 2 * GPC * DQQ], f32, tag="s1e")   # [cot][g][dqq]
        s1p = sb.tile([128, 2 * DQQ], f32, tag="s1p")         # [cot][dqq]
        s2e = sb.tile([128, 2 * GPC * DQQ], f32, tag="s2e")   # [cot][g][dqq]
        s2p = sb.tile([128, 2 * DQQ], f32, tag="s2p")         # [cot][dqq]
        packe = sb.tile([128, 4], f32, tag="packe")
        packp = sb.tile([128, 4], f32, tag="packp")
        globe = sb.tile([128, 4], f32, tag="globe")
        globp = sb.tile([128, 4], f32, tag="globp")

        fpsum = ps_f.tile([128, DW], f32, tag="fpsum")

        # ---- DRAM bounce tiles for collectives -------------------------
        mx_in = dram.tile([128, 2 * L], bf16, tag="mxi")
        mx_out = dram.tile([128, 2 * L], bf16, tag="mxo")
        ste_in = dram.tile([128, 4], f32, tag="stei")
        ste_outs = [dram.tile([128, 4], f32, tag="steo", name=f"ste_out{i}",
                              addr_space="Shared") for i in range(2)]
        warm_in = dram.tile([128, 1], f32, tag="warmi")
        warm_out = dram.tile([128, 1], f32, tag="warmo", addr_space="Shared")
        warm2_in = dram.tile([128, 1], f32, tag="warm2i")
        warm2_out = dram.tile([128, 1], f32, tag="warm2o", addr_space="Shared")
        stp_in = dram.tile([128, 4], f32, tag="stpi")
        stp_outs = [dram.tile([128, 4], f32, tag="stpo", name=f"stp_out{i}",
                              addr_space="Shared") for i in range(2)]

        xsh_d, px_d, ash_d, ashp_d, w_d, gb_d, out_d, out2_d, outst_d = (
            io["xsh"], io["pxsh"], io["Ash"], io["AshP"], io["Wmats"],
            io["gbs"], io["out"], io["out2"], io["outst"])

        # small affine tiles
        eps_t = sb_small.tile([128, 1], f32, tag="sm", name="eps")
        vec.memset(eps_t[:], EPS)
        t0 = sb_small.tile([128, 2], f32, tag="sm", name="t0")
        a1 = sb_small.tile([128, 2], f32, tag="sm", name="a1")
        b1 = sb_small.tile([128, 2], f32, tag="sm", name="b1")
        a2 = sb_small.tile([128, 2], f32, tag="sm", name="a2")
        a2h = sb_small.tile([128, 2], bf16, tag="sm", name="a2h")
        bs = sb_small.tile([128, 2], f32, tag="bs", name="bs")
        me = sb_small.tile([128, 2], f32, tag="sm", name="me")
        ve = sb_small.tile([128, 2], f32, tag="sm", name="ve")


        # ---- input loads: g0's first A-block needs slot0 + Ash[0] only
        sync.dma_start(slots[0][:], xsh_d[0, :, :])
        sync.dma_start(Asb[:, 0:LT * DW], ash_d[:, 0, :])
        sync.dma_start(wsb[0][:], w_d[0, :, :])
        for g in range(1, GPC):
            sync.dma_start(slots[g][:], xsh_d[g, :, :])
        for dqq in range(1, DQQ):
            sync.dma_start(Asb[:, dqq * LT * DW:(dqq + 1) * LT * DW],
                           ash_d[:, dqq, :])
        sync.dma_start(pool_nm[:], px_d[:, :])
        for i in [3, 1, 4, 2, 5]:
            sync.dma_start(wsb[i][:], w_d[i, :, :])
        sync.dma_start(gbt[:], gb_d[:, :])
        # warm-up collective #1: keeps the CC stream busy until every
        # core's layer-1 stats are ready, so AR_e sees no arrival skew.
        sync.dma_start(warm_in[:], eps_t[:])
        _cc(nc, "AllReduce", OP.add, ALL8, warm_in[:], warm_out[:])
        sync.dma_start(Apsb[:, 0:LT * DW], ashp_d[:, 0, :])
        sync.dma_start(Apsb[:, LT * DW:2 * LT * DW], ashp_d[:, 1, :])

        def affine(a_t, b_t, s1_ap, s2_ap, inv_cnt, gslc, beslc):
            # a = g * rsqrt(var+eps); b = be - a*mean
            vec.tensor_scalar(me[:], s1_ap, inv_cnt, None, OP.mult)
            vec.tensor_scalar(ve[:], s2_ap, inv_cnt, None, OP.mult)
            vec.tensor_tensor(t0[:], me[:], me[:], OP.mult)
            vec.tensor_tensor(ve[:], ve[:], t0[:], OP.subtract)
            act.activation(t0[:], ve[:], AF.Sqrt, bias=eps_t[:])
            vec.reciprocal(t0[:], t0[:])
            vec.tensor_tensor(a_t[:], gslc, t0[:], OP.mult)
            vec.tensor_tensor(t0[:], a_t[:], me[:], OP.mult)
            vec.tensor_tensor(b_t[:], beslc, t0[:], OP.subtract)

        def sq_chunk(y_ap, s2_slot):
            """Sum of squares of one [128, DW] drained chunk on DVE."""
            vec.scalar_tensor_tensor(sqscr[:, 0:DW], y_ap, 1.0, y_ap,
                                     OP.mult, OP.mult, accum_out=s2_slot)

        def emit_A_block(src, dqq, drain_dve):
            """L1-style: agg[:, :] = x_chunk^T A[:, dqq block]; one psum."""
            pa = ps_a.tile([128, DW], f32, tag="psa")
            for st in range(LT):
                te.matmul(pa[:], src[:, st * 128:(st + 1) * 128],
                          Asb[:, (dqq * LT + st) * DW:(dqq * LT + st + 1) * DW],
                          start=(st == 0), stop=(st == LT - 1))
            agg = sb_agg.tile([128, DW], bf16, tag="agg")
            if drain_dve:
                vec.tensor_copy(agg[:], pa[:])
            else:
                act.activation(agg[:], pa[:], AF.Copy)
            return agg

        def emit_W_block(li, g, dqq, agg):
            """Project agg (Cin wide) to the two cot halves of y."""
            we = wsb[li] if g < GPC else wsb[3 + li]
            dsty = y1s[g] if g < GPC else y1s[GPC]
            s1 = s1e if g < GPC else s1p
            s2 = s2e if g < GPC else s2p
            for cot in range(2):
                pw = ps_w.tile([128, DW], f32, tag="psw")
                te.matmul(pw[:], we[:, cot * 128:(cot + 1) * 128], agg[:],
                          start=True, stop=True)
                idx = (cot * GPC + g) * DQQ + dqq if g < GPC \
                    else cot * DQQ + dqq
                yap = dsty[:, cot * L + dqq * DW:cot * L + (dqq + 1) * DW]
                act.activation(yap, pw[:], AF.Copy,
                               accum_out=s1[:, idx:idx + 1])
                sq_chunk(yap, s2[:, idx:idx + 1])

        def emit_h(li, g, src):
            """h = src W for L2/L3; paired [128,512] psum, alt ACT/DVE drain."""
            we = wsb[li] if g < GPC else wsb[3 + li]
            h = sb_h.tile([128, LT * 256], bf16, tag="h")
            for sp in range(LT // 2):
                ph = ps_h.tile([128, 512], f32, tag="psh")
                for sub in range(2):
                    st = sp * 2 + sub
                    for ct in range(2):
                        te.matmul(ph[:, sub * 256:(sub + 1) * 256],
                                  src[:, ct * L + st * 128:
                                      ct * L + st * 128 + 128],
                                  we[:, ct * 256:(ct + 1) * 256],
                                  start=(ct == 0), stop=(ct == 1))
                if sp % 2 == 0:
                    act.activation(h[:, sp * 512:(sp + 1) * 512], ph[:],
                                   AF.Copy)
                else:
                    vec.tensor_copy(h[:, sp * 512:(sp + 1) * 512], ph[:])
            return h

        def emit_A(li, g, h, nq=DQQ, asrc=None, out_dram=None):
            """y[cot, dqq] = h^T A for L2/L3 (dqq-major, st accumulation)."""
            if asrc is None:
                asrc = Asb
            dsty = y1s[g] if g < GPC else y1s[GPC]
            s1 = s1e if g < GPC else s1p
            s2 = s2e if g < GPC else s2p
            for cot in range(2):
                for dqq in range(nq):
                    pw = ps_w.tile([128, DW], f32, tag="psw")
                    for st in range(LT):
                        te.matmul(pw[:],
                                  h[:, st * 256 + cot * 128:
                                    st * 256 + cot * 128 + 128],
                                  asrc[:, (dqq * LT + st) * DW:
                                       (dqq * LT + st + 1) * DW],
                                  start=(st == 0), stop=(st == LT - 1))
                    if g < GPC:
                        idx = (cot * GPC + g) * DQQ + dqq
                        col = cot * L + dqq * DW
                    else:
                        idx = cot * nq + dqq
                        col = cot * nq * DW + dqq * DW
                    yap = dsty[:, col:col + DW]
                    act.activation(yap, pw[:], AF.Copy,
                                   accum_out=s1[:, idx:idx + 1])
                    sq_chunk(yap, s2[:, idx:idx + 1])
                    if out_dram is not None:
                        sync.dma_start(out_dram[:, col:col + DW], yap)

        def pack_launch_e(li):
            for cot in range(2):
                vec.reduce_sum(packe[:, cot:cot + 1],
                               s1e[:, cot * GPC * DQQ:(cot + 1) * GPC * DQQ],
                               axis=AX.X)
                vec.reduce_sum(packe[:, 2 + cot:3 + cot],
                               s2e[:, cot * GPC * DQQ:(cot + 1) * GPC * DQQ],
                               axis=AX.X)
            sync.dma_start(ste_in[:], packe[:])
            _cc(nc, "AllReduce", OP.add, ALL8, ste_in[:], ste_outs[li][:])
            sync.dma_start(globe[:], ste_outs[li][:])

        def pack_launch_p(li, nq=DQQ):
            for cot in range(2):
                vec.reduce_sum(packp[:, cot:cot + 1],
                               s1p[:, cot * nq:(cot + 1) * nq], axis=AX.X)
                vec.reduce_sum(packp[:, 2 + cot:3 + cot],
                               s2p[:, cot * nq:(cot + 1) * nq], axis=AX.X)
            sync.dma_start(stp_in[:], packp[:])
            _cc(nc, "AllReduce", OP.add, ALL8, stp_in[:], stp_outs[li][:])
            sync.dma_start(globp[:], stp_outs[li][:])

        def prescale(li):
            # during AR_p flight: affine_e then t = a1*y1 + (b1+b2)... but
            # b2 needs pooled stats; use t = a1*y1 + b1 and fold b2 into the
            # pooled-side bias after AR_p.
            affine(a1, b1, globe[:, 0:2], globe[:, 2:4], 1.0 / CNT_E,
                   gbt[:, 4 * li:4 * li + 2], gbt[:, 4 * li + 2:4 * li + 4])
            for g in range(GPC):
                for cot in range(2):
                    vec.tensor_scalar(
                        y1s[g][:, cot * L:(cot + 1) * L],
                        y1s[g][:, cot * L:(cot + 1) * L],
                        a1[:, cot:cot + 1], b1[:, cot:cot + 1],
                        OP.mult, OP.add)

        def y2v_scale(nchunks=2):
            # v = a2*y2 + b2 into poolb (free at the boundary); phase3 then
            # only needs an add + relu per copy.
            cl = L // nchunks
            for ch in range(nchunks):
                for cot in range(2):
                    sl = slice(cot * L + ch * cl, cot * L + (ch + 1) * cl)
                    vec.tensor_scalar(poolb[:, sl], y1s[GPC][:, sl],
                                      a2[:, cot:cot + 1], bs[:, cot:cot + 1],
                                      OP.mult, OP.add)

        def phase3_g(li, g, nchunks=1):
            # x'_g = relu(t_g + v);  t = a1*y1+b1 (prescaled), v in poolb
            cl = L // nchunks
            for ch in range(nchunks):
                for cot in range(2):
                    sl = slice(cot * L + ch * cl, cot * L + (ch + 1) * cl)
                    vec.tensor_tensor(y1s[g][:, sl], poolb[:, sl],
                                      y1s[g][:, sl], OP.add)
                    act.activation(y1s[g][:, sl], y1s[g][:, sl], AF.Relu)

        def fillers(n):
            # junk matmuls that keep the PE HAM clock-gate warm across a
            # collective wait; never read back.  The rhs reads the pooled
            # instance's last-drained chunk so the scheduler cannot hoist
            # the batch away from the layer boundary it must cover.
            for _ in range(n):
                te.matmul(fpsum[:], wsb[0][:, 0:128],
                          y1s[GPC][:, 2 * L - DW:2 * L],
                          start=True, stop=True)

        # ================= LAYER 1 (agg-first, dqq-outer) ================
        pendW = None           # (g, dqq, agg) carried one block behind
        for dqq in range(DQQ):
            for g in range(GPC):
                agg = emit_A_block(slots[g], dqq, drain_dve=(g % 2 == 1))
                if pendW is not None:
                    emit_W_block(0, pendW[0], pendW[1], pendW[2])
                pendW = (g, dqq, agg)
        emit_W_block(0, pendW[0], pendW[1], pendW[2])
        pack_launch_e(0)

        # pooled L1 instance (hides AR_e)
        pendW = None
        for dqq in range(DQQ):
            agg = emit_A_block(pool_nm, dqq, drain_dve=(dqq % 2 == 1))
            if pendW is not None:
                emit_W_block(0, GPC, pendW[1], pendW[2])
            pendW = (GPC, dqq, agg)
        emit_W_block(0, GPC, pendW[1], pendW[2])
        pack_launch_p(0)
        fillers(_NFILL)
        prescale(0)

        # ================= LAYERS 2..3 ===================================
        for li in (1, 2):
            last = (li == 2)
            # boundary: affine_p, then per-copy phase3 feeding this layer
            affine(a2, bs, globp[:, 0:2], globp[:, 2:4], 1.0 / CNT_P,
                   gbt[:, 12 + 4 * (li - 1):14 + 4 * (li - 1)],
                   gbt[:, 14 + 4 * (li - 1):16 + 4 * (li - 1)])
            y2v_scale()

            hs = [None] * (GPC + 1)
            phase3_g(li, 0, nchunks=2)
            hs[0] = emit_h(li, 0, y1s[0])
            if not last:
                for g in range(1, GPC):
                    if g == 2:
                        # warm-up collective #2: re-sync cores mid-layer so
                        # the boundary AllReduces see minimal arrival skew
                        # (anchored on this layer's first accum slot).
                        sync.dma_start(warm2_in[:], s1e[:, 0:1])
                        _cc(nc, "AllReduce", OP.add, ALL8,
                            warm2_in[:], warm2_out[:])
                    phase3_g(li, g)
                    if g == 1:
                        vec.tensor_tensor(mxsb[:], y1s[0][:], y1s[1][:],
                                          OP.max)
                    else:
                        vec.tensor_tensor(mxsb[:], mxsb[:], y1s[g][:], OP.max)
                    emit_A(li, g - 1, hs[g - 1])
                    hs[g] = emit_h(li, g, y1s[g])
                    if g == GPC - 1:
                        # pool-max exchange feeding THIS layer's pooled inst
                        sync.dma_start(mx_in[:], mxsb[:])
                        _cc(nc, "AllReduce", OP.max, PAIRS,
                            mx_in[:], mx_out[:])
                        sync.dma_start(poolb[:], mx_out[:])
                emit_A(li, GPC - 1, hs[GPC - 1])
                pack_launch_e(1)
                # pooled instance (hides AR_e)
                hs[GPC] = emit_h(li, GPC, poolb)
                emit_A(li, GPC, hs[GPC])
                pack_launch_p(1)
                fillers(_NFILL)
                prescale(li)
            else:
                # L3 tensor order: h0 A0 h1 A1 h2 A2 hp Ap h3 A3; pre-BN
                # y1/y2 halves + stat sums stream out, host finishes BN+relu.
                phase3_g(li, 1)
                vec.tensor_tensor(mxsb[:], y1s[0][:], y1s[1][:], OP.max)
                emit_A(li, 0, hs[0], out_dram=out_d[0, :, :])
                hs[1] = emit_h(li, 1, y1s[1])
                phase3_g(li, 2)
                vec.tensor_tensor(mxsb[:], mxsb[:], y1s[2][:], OP.max)
                emit_A(li, 1, hs[1], out_dram=out_d[1, :, :])
                hs[2] = emit_h(li, 2, y1s[2])
                phase3_g(li, 3)
                vec.tensor_tensor(mxsb[:], mxsb[:], y1s[3][:], OP.max)
                sync.dma_start(mx_in[:], mxsb[:])
                _cc(nc, "AllReduce", OP.max, PAIRS, mx_in[:], mx_out[:])
                sync.dma_start(poolb[:], mx_out[:])
                emit_A(li, 2, hs[2], out_dram=out_d[2, :, :])
                hs[GPC] = emit_h(li, GPC, poolb)
                emit_A(li, GPC, hs[GPC], nq=2, asrc=Apsb,
                       out_dram=out2_d)
                for cot in range(2):
                    vec.reduce_sum(packp[:, cot:cot + 1],
                                   s1p[:, cot * 2:(cot + 1) * 2], axis=AX.X)
                    vec.reduce_sum(packp[:, 2 + cot:3 + cot],
                                   s2p[:, cot * 2:(cot + 1) * 2], axis=AX.X)
                sync.dma_start(outst_d[:, 4:8], packp[:])
                hs[3] = emit_h(li, 3, y1s[3])
                emit_A(li, 3, hs[3], out_dram=out_d[3, :, :])
                for cot in range(2):
                    vec.reduce_sum(packe[:, cot:cot + 1],
                                   s1e[:, cot * GPC * DQQ:
                                       (cot + 1) * GPC * DQQ], axis=AX.X)
                    vec.reduce_sum(packe[:, 2 + cot:3 + cot],
                                   s2e[:, cot * GPC * DQQ:
                                       (cot + 1) * GPC * DQQ], axis=AX.X)
                sync.dma_start(outst_d[:, 0:4], packe[:])


def _build():
    key = ("nc", _NO_CC)
    if key in _CACHE:
        return _CACHE[key]
    nc = bacc.Bacc("TRN2", target_bir_lowering=False, debug=False,
                   num_devices=NCORES)
    io = {
        "xsh": nc.dram_tensor("xsh", [GPC, 128, LT * 128], bf16,
                              kind="ExternalInput"),
        "pxsh": nc.dram_tensor("pxsh", [128, LT * 128], bf16,
                               kind="ExternalInput"),
        "Ash": nc.dram_tensor("Ash", [128, DQQ, LT * DW], bf16,
                              kind="ExternalInput"),
        "AshP": nc.dram_tensor("AshP", [128, 2, LT * DW], bf16,
                               kind="ExternalInput"),
        "Wmats": nc.dram_tensor("Wmats", [6, 128, 512], bf16,
                                kind="ExternalInput"),
        "gbs": nc.dram_tensor("gbs", [128, 24], f32, kind="ExternalInput"),
        "out": nc.dram_tensor("out", [GPC, 128, 2 * L], bf16,
                              kind="ExternalOutput"),
        "out2": nc.dram_tensor("out2", [128, 4 * DW], bf16,
                               kind="ExternalOutput"),
        "outst": nc.dram_tensor("outst", [128, 8], f32,
                                kind="ExternalOutput"),
    }
    with tile.TileContext(nc) as tc:
        _emit(tc, nc, io)
    nc.compile()
    _CACHE[key] = nc
    return nc


def _bf16(a):
    return np.asarray(a, np.float32).astype(ml_dtypes.bfloat16)


def _host_prep(edge_index, Ws, gs, bes):
    """Build the device-layout arrays on host."""
    src = np.asarray(edge_index[0], dtype=np.int64)
    dst = np.asarray(edge_index[1], dtype=np.int64)
    deg = np.zeros(L, np.float32)
    np.add.at(deg, dst, np.float32(1.0))
    deg += np.float32(2.0)
    dis = (1.0 / np.sqrt(deg.astype(np.float64))).astype(np.float32)
    A = np.zeros((L, L), np.float32)
    np.add.at(A, (src, dst), dis[src] * dis[dst])
    A[np.arange(L), np.arange(L)] += np.float32(2.0) * dis * dis
    ash = _bf16(np.ascontiguousarray(
        A.reshape(LT, 128, DQQ, DW).transpose(1, 2, 0, 3).reshape(128, DQQ, LT * DW)))

    wm = np.zeros((6, 128, 512), np.float32)
    for i, W in enumerate(Ws):
        cin = W.shape[0]
        wm[i, :, : (cin // 128) * 256] = np.ascontiguousarray(
            W.reshape(cin // 128, 128, 256).transpose(1, 0, 2).reshape(128, -1))
    wm = _bf16(wm)

    gb = np.zeros((128, 24), np.float32)
    vecs = [gs[0], bes[0], gs[1], bes[1], gs[2], bes[2],
            gs[3], bes[3], gs[4], bes[4], gs[5], bes[5]]
    for v, w in enumerate(vecs):
        gb[:, v * 2 + 0] = w[0:128]
        gb[:, v * 2 + 1] = w[128:256]
    return ash, wm, gb


def kernel(x, edge_index, W1, b1, W2, b2, W3, b3, W1s, b1s, W2s, b2s, W3s, b3s,
           g1, be1, g2, be2, g3, be3, g1s, be1s, g2s, be2s, g3s, be3s):
    x = np.asarray(x, np.float32)
    ash, wm, gb = _host_prep(
        np.asarray(edge_index),
        [np.asarray(W1, np.float32), np.asarray(W2, np.float32),
         np.asarray(W3, np.float32), np.asarray(W1s, np.float32),
         np.asarray(W2s, np.float32), np.asarray(W3s, np.float32)],
        [np.asarray(g1, np.float32), np.asarray(g2, np.float32),
         np.asarray(g3, np.float32), np.asarray(g1s, np.float32),
         np.asarray(g2s, np.float32), np.asarray(g3s, np.float32)],
        [np.asarray(be1, np.float32), np.asarray(be2, np.float32),
         np.asarray(be3, np.float32), np.asarray(be1s, np.float32),
         np.asarray(be2s, np.float32), np.asarray(be3s, np.float32)])

    # core k: graph b=k//2, copies n in [4*(k%2), 4*(k%2)+4)
    # upload x node-major: slot[p, st*128 + c] = x[st*128+p, c]
    xr = x.reshape(NCORES, GPC, CH[0], L)
    xnm = _bf16(np.ascontiguousarray(
        xr.reshape(NCORES, GPC, 128, LT, 128).transpose(0, 1, 4, 3, 2)
          .reshape(NCORES, GPC, 128, LT * 128)))
    # pooled-branch input: max over the 8 copies of each graph, node-major
    xp = x.reshape(B, N, CH[0], L).max(axis=1)  # [B, 128, L]
    xpnm = _bf16(np.ascontiguousarray(
        xp.reshape(B, 128, LT, 128).transpose(0, 3, 2, 1)
          .reshape(B, 128, LT * 128)))
    in_maps = []
    for k in range(NCORES):
        par = k % 2
        in_maps.append({
            "xsh": xnm[k], "pxsh": xpnm[k // 2],
            "Ash": ash, "AshP": np.ascontiguousarray(ash[:, 2 * par:2 * par + 2, :]),
            "Wmats": wm, "gbs": gb,
        })

    nc = _build()

    if _SIMULATE:
        from concourse.bass_interp import MultiCoreSim
        sim = MultiCoreSim(nc, NCORES)
        for k in range(NCORES):
            for nm, arr in in_maps[k].items():
                sim.cores[k].tensor(nm)[:] = arr
        sim.simulate(check_with_hw=False)
        outs = [np.array(sim.cores[k].mem_tensor("out")) for k in range(NCORES)]
        outs2 = [np.array(sim.cores[k].mem_tensor("out2")) for k in range(NCORES)]
        outsst = [np.array(sim.cores[k].mem_tensor("outst")) for k in range(NCORES)]
    else:
        res = run_bass_kernel_spmd(nc, in_maps, core_ids=list(range(NCORES)),
                                   trace=_PROFILE)
        if _PROFILE:
            _CACHE["last_result"] = res
        outs = [np.asarray(res.results[k]["out"]) for k in range(NCORES)]
        outs2 = [np.asarray(res.results[k]["out2"]) for k in range(NCORES)]
        outsst = [np.asarray(res.results[k]["outst"]) for k in range(NCORES)]

    # ---- host-side final layer: BN affine + pooled add + relu ----------
    # buf [GPC, 128, 2*L] bf16: y[g, cot*128+p, n] = buf[g, p, cot*L+n]
    y1 = np.stack([o.astype(np.float32) for o in outs])        # [8,G,128,2L]
    y1 = (y1.reshape(NCORES, GPC, 128, 2, L).transpose(0, 1, 3, 2, 4)
            .reshape(NCORES, GPC, 256, L))
    # out2 halves: core 2b+par holds dest-node cols (2*par+d2)*512+j
    y2 = np.empty((B, 256, L), np.float32)
    for b in range(B):
        for par in range(2):
            h = outs2[2 * b + par].astype(np.float32)  # [128, 4*DW]
            h = h.reshape(128, 2, 2, DW)               # [p, cot, d2, j]
            for cot in range(2):
                for d2 in range(2):
                    nd = (2 * par + d2) * DW
                    y2[b, cot * 128:(cot + 1) * 128, nd:nd + DW] = \
                        h[:, cot, d2, :]
    st = np.sum(np.stack([o.astype(np.float64) for o in outsst]), axis=0)

    def bn_affine(s1, s2, cnt, g, be):
        m = s1 / cnt
        v = s2 / cnt - m * m
        a = np.asarray(g, np.float64) / np.sqrt(v + EPS)
        return (a.astype(np.float32),
                (np.asarray(be, np.float64) - a * m).astype(np.float32))

    a1, b1 = bn_affine(st[:, 0:2].T.reshape(256), st[:, 2:4].T.reshape(256),
                       CNT_E, g3, be3)
    a2, b2 = bn_affine(st[:, 4:6].T.reshape(256), st[:, 6:8].T.reshape(256),
                       CNT_P3, g3s, be3s)
    bsum = (b1 + b2)[None, :, None]
    out = np.empty((NCORES * GPC, 256, L), np.float32)
    for k in range(NCORES):
        out[k * GPC:(k + 1) * GPC] = (a1[None, :, None] * y1[k]
                                      + a2[None, :, None] * y2[k // 2] + bsum)
    np.maximum(out, 0.0, out=out)
    return out


# revision 19
# speedup vs baseline: 1.0407x; 1.0229x over previous
"""Trainium2 Bass kernel for nn_DeepSymmetricGCN1dBlock.

3-layer GCN block over a shared 2048-node graph, 32 graph copies (b=4, n=8),
channels 128->256->256->256, per-element branch + symmetric max-pooled branch,
training-mode BatchNorm, ReLU.

Strategy (v5)
-------------
Data-parallel over the 32 graph copies: core k holds copies of graph b=k//2,
n in [4*(k%2), 4*(k%2)+4).  The sparse GCN aggregation is a dense matmul
against the normalized adjacency A_hat [2048, 2048], kept RESIDENT in SBUF
in bf16 (8 MiB), streamed in N=512 moving chunks.  All matmul operands are
bf16 (PSUM accumulation stays fp32); BN statistics are fp32.

Layer 1 runs aggregation-first (agg = x^T A at Cin=128 width; x is uploaded
pre-transposed to node-major by the host), dqq-OUTER across the 4 element
instances so compute starts as soon as the first A chunk lands.  Layers 2-3
run W-first (h = x W, then y = h^T A); h psum is drained in [128,512] pairs
alternating ACT/DVE so drains keep up with the matmul stream.

Per layer: 4 element instances first, then element-stats AllReduce (hidden
under the pooled instance), then pooled-stats AllReduce.  During the pooled
AR flight the element BN affine is pre-applied in place
(t = a1*y1 + (b1+b2), DVE), so post-AR work is just
x' = relu(a2*y2 + t) per copy.  Pool-max AllReduce runs in bf16 (exact) and
lands during the next layer's element matmuls.  Layer 3 ships pre-BN y1/y2
plus stat sums; its pooled instance is pair-split by destination-node halves
(per-core Ash_pool input selects the half) and the host stitches + applies
the final BN affine + relu.
"""

import sys

if "/opt/trn_rl_repo" not in sys.path:
    sys.path.insert(0, "/opt/trn_rl_repo")

import numpy as np
import ml_dtypes

import concourse.bass as bass
import concourse.bacc as bacc
import concourse.mybir as mybir
import concourse.tile as tile
from concourse.bass_utils import run_bass_kernel_spmd

f32 = mybir.dt.float32
bf16 = mybir.dt.bfloat16
AF = mybir.ActivationFunctionType
OP = mybir.AluOpType
AX = mybir.AxisListType

B, N, L, E = 4, 8, 2048, 16384
CH = [128, 256, 256, 256]
EPS = 1e-5
NCORES = 8
GPC = 4            # graph copies per core
LT = L // 128      # 16 node tiles
DQQ = 4            # A streamed in DQQ chunks of DW destination columns
DW = L // DQQ      # 512
CNT_E = 32 * L     # element-branch BN count (all 32 copies)
CNT_P = 8 * L      # pooled-branch BN count L1/L2 (4 graphs, pair-redundant)
CNT_P3 = 4 * L     # pooled L3: pair-split, each node counted once

PAIRS = [[0, 1], [2, 3], [4, 5], [6, 7]]
ALL8 = [list(range(NCORES))]

import os
_PROFILE = False
_SIMULATE = False
_NO_CC = os.environ.get("K_NO_CC", "0") == "1"
_NFILL = int(os.environ.get("K_NFILL", "70"))
_CACHE = {}


def _cc(nc, kind, op, groups, bi, bo):
    """bi/bo are APs into DRAM bounce tiles."""
    if _NO_CC:
        nc.sync.dma_start(bo, bi)
    else:
        nc.gpsimd.collective_compute(kind, op, replica_groups=groups,
                                     ins=[bi.opt()], outs=[bo.opt()])


def _emit(tc, nc, io):
    sync, vec, act, te = nc.sync, nc.vector, nc.scalar, nc.tensor

    from contextlib import ExitStack

    ctx = ExitStack()
    with ctx:
        sb = ctx.enter_context(tc.tile_pool(name="sb", bufs=1))
        sb_slot = ctx.enter_context(tc.tile_pool(name="slots", bufs=GPC + 1))
        sb_y1 = ctx.enter_context(tc.tile_pool(name="y1", bufs=GPC + 2))
        sb_h = ctx.enter_context(tc.tile_pool(name="h", bufs=2))
        sb_agg = ctx.enter_context(tc.tile_pool(name="agg", bufs=3))
        sb_w = ctx.enter_context(tc.tile_pool(name="w", bufs=6))
        sb_small = ctx.enter_context(tc.tile_pool(name="small", bufs=28))
        ps_a = ctx.enter_context(tc.tile_pool(name="psa", bufs=2, space="PSUM"))
        ps_w = ctx.enter_context(tc.tile_pool(name="psw", bufs=2, space="PSUM"))
        ps_h = ctx.enter_context(tc.tile_pool(name="psh", bufs=2, space="PSUM"))
        ps_f = ctx.enter_context(tc.tile_pool(name="psf", bufs=1, space="PSUM"))
        dram = ctx.enter_context(tc.tile_pool(name="dram", bufs=1, space="DRAM"))

        # ---- persistent SBUF tiles -------------------------------------
        Asb = sb.tile([128, DQQ * LT * DW], bf16, tag="Asb")
        Apsb = sb.tile([128, 2 * LT * DW], bf16, tag="Apsb")  # L3 pool half
        slots = [sb_slot.tile([128, LT * 128], bf16, tag="slot", name=f"slot{i}")
                 for i in range(GPC)]
        pool_nm = sb_slot.tile([128, LT * 128], bf16, tag="slot", name="pool_nm")
        y1s = [sb_y1.tile([128, 2 * L], bf16, tag="y1", name=f"y1_{i}")
               for i in range(GPC + 1)]
        poolb = sb_y1.tile([128, 2 * L], bf16, tag="y1", name="poolb")
        mxsb = sb.tile([128, 2 * L], bf16, tag="mxsb")
        sqscr = sb.tile([128, L], bf16, tag="sqscr")
        wsb = [sb_w.tile([128, 512], bf16, tag="w", name=f"w{i}") for i in range(6)]
        gbt = sb.tile([128, 24], f32, tag="gbt")
        s1e = sb.tile([128, 2 * GPC * DQQ], f32, tag="s1e")   # [cot][g][dqq]
        s1p = sb.tile([128, 2 * DQQ], f32, tag="s1p")         # [cot][dqq]
        s2e = sb.tile([128, 2 * GPC * DQQ], f32, tag="s2e")   # [cot][g][dqq]
        s2p = sb.tile([128, 2 * DQQ], f32, tag="s2p")         # [cot][dqq]
        packe = sb.tile([128, 4], f32, tag="packe")
        packp = sb.tile([128, 4], f32, tag="packp")
        globe = sb.tile([128, 4], f32, tag="globe")
        globp = sb.tile([128, 4], f32, tag="globp")

        fpsum = ps_f.tile([128, DW], f32, tag="fpsum")

        # ---- DRAM bounce tiles for collectives -------------------------
        mx_in = dram.tile([128, 2 * L], bf16, tag="mxi")
        mx_out = dram.tile([128, 2 * L], bf16, tag="mxo")
        ste_in = dram.tile([128, 4], f32, tag="stei")
        ste_outs = [dram.tile([128, 4], f32, tag="steo", name=f"ste_out{i}",
                              addr_space="Shared") for i in range(2)]
        warm_in = dram.tile([128, 1], f32, tag="warmi")
        warm_out = dram.tile([128, 1], f32, tag="warmo", addr_space="Shared")
        warm2_in = dram.tile([128, 1], f32, tag="warm2i")
        warm2_out = dram.tile([128, 1], f32, tag="warm2o", addr_space="Shared")
        stp_in = dram.tile([128, 4], f32, tag="stpi")
        stp_outs = [dram.tile([128, 4], f32, tag="stpo", name=f"stp_out{i}",
                              addr_space="Shared") for i in range(2)]

        xsh_d, px_d, ash_d, ashp_d, w_d, gb_d, out_d, out2_d, outst_d = (
            io["xsh"], io["pxsh"], io["Ash"], io["AshP"], io["Wmats"],
            io["gbs"], io["out"], io["out2"], io["outst"])

        # small affine tiles
        eps_t = sb_small.tile([128, 1], f32, tag="sm", name="eps")
        vec.memset(eps_t[:], EPS)
        t0 = sb_small.tile([128, 2], f32, tag="sm", name="t0")
        a1 = sb_small.tile([128, 2], f32, tag="sm", name="a1")
        b1 = sb_small.tile([128, 2], f32, tag="sm", name="b1")
        a2 = sb_small.tile([128, 2], f32, tag="sm", name="a2")
        a2h = sb_small.tile([128, 2], bf16, tag="sm", name="a2h")
        bs = sb_small.tile([128, 2], f32, tag="bs", name="bs")
        me = sb_small.tile([128, 2], f32, tag="sm", name="me")
        ve = sb_small.tile([128, 2], f32, tag="sm", name="ve")


        # ---- input loads: the L1 pooled instance runs first, so its
        # inputs (pool_nm + Ash chunk 0, in small pieces) lead the queue.
        for p in range(4):
            sync.dma_start(pool_nm[:, p * 512:(p + 1) * 512],
                           px_d[:, p * 512:(p + 1) * 512])
        for p in range(4):
            sync.dma_start(Asb[:, p * 2048:(p + 1) * 2048],
                           ash_d[:, 0, p * 2048:(p + 1) * 2048])
        sync.dma_start(wsb[3][:], w_d[3, :, :])
        sync.dma_start(slots[0][:], xsh_d[0, :, :])
        sync.dma_start(wsb[0][:], w_d[0, :, :])
        sync.dma_start(Asb[:, LT * DW:2 * LT * DW], ash_d[:, 1, :])
        for g in range(1, GPC):
            sync.dma_start(slots[g][:], xsh_d[g, :, :])
        for dqq in range(2, DQQ):
            sync.dma_start(Asb[:, dqq * LT * DW:(dqq + 1) * LT * DW],
                           ash_d[:, dqq, :])
        for i in [1, 4, 2, 5]:
            sync.dma_start(wsb[i][:], w_d[i, :, :])
        sync.dma_start(gbt[:], gb_d[:, :])
        sync.dma_start(Apsb[:, 0:LT * DW], ashp_d[:, 0, :])
        sync.dma_start(Apsb[:, LT * DW:2 * LT * DW], ashp_d[:, 1, :])

        def affine(a_t, b_t, s1_ap, s2_ap, inv_cnt, gslc, beslc):
            # a = g * rsqrt(var+eps); b = be - a*mean
            vec.tensor_scalar(me[:], s1_ap, inv_cnt, None, OP.mult)
            vec.tensor_scalar(ve[:], s2_ap, inv_cnt, None, OP.mult)
            vec.tensor_tensor(t0[:], me[:], me[:], OP.mult)
            vec.tensor_tensor(ve[:], ve[:], t0[:], OP.subtract)
            act.activation(t0[:], ve[:], AF.Sqrt, bias=eps_t[:])
            vec.reciprocal(t0[:], t0[:])
            vec.tensor_tensor(a_t[:], gslc, t0[:], OP.mult)
            vec.tensor_tensor(t0[:], a_t[:], me[:], OP.mult)
            vec.tensor_tensor(b_t[:], beslc, t0[:], OP.subtract)

        def sq_chunk(y_ap, s2_slot):
            """Sum of squares of one [128, DW] drained chunk on DVE."""
            vec.scalar_tensor_tensor(sqscr[:, 0:DW], y_ap, 1.0, y_ap,
                                     OP.mult, OP.mult, accum_out=s2_slot)

        def emit_A_block(src, dqq, drain_dve):
            """L1-style: agg[:, :] = x_chunk^T A[:, dqq block]; one psum."""
            pa = ps_a.tile([128, DW], f32, tag="psa")
            for st in range(LT):
                te.matmul(pa[:], src[:, st * 128:(st + 1) * 128],
                          Asb[:, (dqq * LT + st) * DW:(dqq * LT + st + 1) * DW],
                          start=(st == 0), stop=(st == LT - 1))
            agg = sb_agg.tile([128, DW], bf16, tag="agg")
            if drain_dve:
                vec.tensor_copy(agg[:], pa[:])
            else:
                act.activation(agg[:], pa[:], AF.Copy)
            return agg

        def emit_W_block(li, g, dqq, agg):
            """Project agg (Cin wide) to the two cot halves of y."""
            we = wsb[li] if g < GPC else wsb[3 + li]
            dsty = y1s[g] if g < GPC else y1s[GPC]
            s1 = s1e if g < GPC else s1p
            s2 = s2e if g < GPC else s2p
            for cot in range(2):
                pw = ps_w.tile([128, DW], f32, tag="psw")
                te.matmul(pw[:], we[:, cot * 128:(cot + 1) * 128], agg[:],
                          start=True, stop=True)
                idx = (cot * GPC + g) * DQQ + dqq if g < GPC \
                    else cot * DQQ + dqq
                yap = dsty[:, cot * L + dqq * DW:cot * L + (dqq + 1) * DW]
                act.activation(yap, pw[:], AF.Copy,
                               accum_out=s1[:, idx:idx + 1])
                sq_chunk(yap, s2[:, idx:idx + 1])

        def emit_h(li, g, src):
            """h = src W for L2/L3; paired [128,512] psum, alt ACT/DVE drain."""
            we = wsb[li] if g < GPC else wsb[3 + li]
            h = sb_h.tile([128, LT * 256], bf16, tag="h")
            for sp in range(LT // 2):
                ph = ps_h.tile([128, 512], f32, tag="psh")
                for sub in range(2):
                    st = sp * 2 + sub
                    for ct in range(2):
                        te.matmul(ph[:, sub * 256:(sub + 1) * 256],
                                  src[:, ct * L + st * 128:
                                      ct * L + st * 128 + 128],
                                  we[:, ct * 256:(ct + 1) * 256],
                                  start=(ct == 0), stop=(ct == 1))
                if sp % 2 == 0:
                    act.activation(h[:, sp * 512:(sp + 1) * 512], ph[:],
                                   AF.Copy)
                else:
                    vec.tensor_copy(h[:, sp * 512:(sp + 1) * 512], ph[:])
            return h

        def emit_A(li, g, h, nq=DQQ, asrc=None, out_dram=None):
            """y[cot, dqq] = h^T A for L2/L3 (dqq-major, st accumulation)."""
            if asrc is None:
                asrc = Asb
            dsty = y1s[g] if g < GPC else y1s[GPC]
            s1 = s1e if g < GPC else s1p
            s2 = s2e if g < GPC else s2p
            for cot in range(2):
                for dqq in range(nq):
                    pw = ps_w.tile([128, DW], f32, tag="psw")
                    for st in range(LT):
                        te.matmul(pw[:],
                                  h[:, st * 256 + cot * 128:
                                    st * 256 + cot * 128 + 128],
                                  asrc[:, (dqq * LT + st) * DW:
                                       (dqq * LT + st + 1) * DW],
                                  start=(st == 0), stop=(st == LT - 1))
                    if g < GPC:
                        idx = (cot * GPC + g) * DQQ + dqq
                        col = cot * L + dqq * DW
                    else:
                        idx = cot * nq + dqq
                        col = cot * nq * DW + dqq * DW
                    yap = dsty[:, col:col + DW]
                    act.activation(yap, pw[:], AF.Copy,
                                   accum_out=s1[:, idx:idx + 1])
                    sq_chunk(yap, s2[:, idx:idx + 1])
                    if out_dram is not None:
                        sync.dma_start(out_dram[:, col:col + DW], yap)

        def pack_launch_e(li):
            for cot in range(2):
                vec.reduce_sum(packe[:, cot:cot + 1],
                               s1e[:, cot * GPC * DQQ:(cot + 1) * GPC * DQQ],
                               axis=AX.X)
                vec.reduce_sum(packe[:, 2 + cot:3 + cot],
                               s2e[:, cot * GPC * DQQ:(cot + 1) * GPC * DQQ],
                               axis=AX.X)
            sync.dma_start(ste_in[:], packe[:])
            _cc(nc, "AllReduce", OP.add, ALL8, ste_in[:], ste_outs[li][:])
            sync.dma_start(globe[:], ste_outs[li][:])

        def pack_launch_p(li, nq=DQQ):
            for cot in range(2):
                vec.reduce_sum(packp[:, cot:cot + 1],
                               s1p[:, cot * nq:(cot + 1) * nq], axis=AX.X)
                vec.reduce_sum(packp[:, 2 + cot:3 + cot],
                               s2p[:, cot * nq:(cot + 1) * nq], axis=AX.X)
            sync.dma_start(stp_in[:], packp[:])
            _cc(nc, "AllReduce", OP.add, ALL8, stp_in[:], stp_outs[li][:])
            sync.dma_start(globp[:], stp_outs[li][:])

        def affine_e(li):
            affine(a1, b1, globe[:, 0:2], globe[:, 2:4], 1.0 / CNT_E,
                   gbt[:, 4 * li:4 * li + 2], gbt[:, 4 * li + 2:4 * li + 4])

        def affine_p(pl, cnt):
            # pl = pooled layer index (0-based); writes a2/bs
            affine(a2, bs, globp[:, 0:2], globp[:, 2:4], 1.0 / cnt,
                   gbt[:, 12 + 4 * pl:14 + 4 * pl],
                   gbt[:, 14 + 4 * pl:16 + 4 * pl])

        def prescale_g(g):
            # t = a1*y1 + b1 in place (element-BN pre-application)
            for cot in range(2):
                vec.tensor_scalar(
                    y1s[g][:, cot * L:(cot + 1) * L],
                    y1s[g][:, cot * L:(cot + 1) * L],
                    a1[:, cot:cot + 1], b1[:, cot:cot + 1],
                    OP.mult, OP.add)

        def prescale(li):
            affine_e(li)
            for g in range(GPC):
                prescale_g(g)

        def y2v_scale(nchunks=2):
            # v = a2*y2 + b2 into poolb (free at the boundary); phase3 then
            # only needs an add + relu per copy.
            cl = L // nchunks
            for ch in range(nchunks):
                for cot in range(2):
                    sl = slice(cot * L + ch * cl, cot * L + (ch + 1) * cl)
                    vec.tensor_scalar(poolb[:, sl], y1s[GPC][:, sl],
                                      a2[:, cot:cot + 1], bs[:, cot:cot + 1],
                                      OP.mult, OP.add)

        def phase3_g(li, g, nchunks=1):
            # x'_g = relu(t_g + v);  t = a1*y1+b1 (prescaled), v in poolb
            cl = L // nchunks
            for ch in range(nchunks):
                for cot in range(2):
                    sl = slice(cot * L + ch * cl, cot * L + (ch + 1) * cl)
                    vec.tensor_tensor(y1s[g][:, sl], poolb[:, sl],
                                      y1s[g][:, sl], OP.add)
                    act.activation(y1s[g][:, sl], y1s[g][:, sl], AF.Relu)

        def fillers(n, anchor):
            # junk matmuls that keep the PE HAM clock-gate warm across a
            # collective wait; never read back.  The rhs reads the batch's
            # anchor (the last-drained chunk before the boundary) so the
            # scheduler cannot hoist it away; the 2-column stationary keeps
            # the PE "busy" for HAM at ~1/64 the energy of a full matmul.
            for _ in range(n):
                te.matmul(fpsum[0:2, :], wsb[0][:, 0:2], anchor,
                          start=True, stop=True)

        # ================= LAYER 1 (agg-first, dqq-outer) ================
        # pooled instance FIRST (input is host-provided): its stats
        # AllReduce flies while the element instances run, fully hidden.
        pendW = None
        for dqq in range(DQQ):
            agg = emit_A_block(pool_nm, dqq, drain_dve=(dqq % 2 == 1))
            if pendW is not None:
                emit_W_block(0, GPC, pendW[1], pendW[2])
            pendW = (GPC, dqq, agg)
        emit_W_block(0, GPC, pendW[1], pendW[2])
        pack_launch_p(0)

        pendW = None           # (g, dqq, agg) carried one block behind
        for dqq in range(DQQ):
            for g in range(GPC):
                agg = emit_A_block(slots[g], dqq, drain_dve=(g % 2 == 1))
                if pendW is not None:
                    emit_W_block(0, pendW[0], pendW[1], pendW[2])
                pendW = (g, dqq, agg)
        emit_W_block(0, pendW[0], pendW[1], pendW[2])
        pack_launch_e(0)
        # pooled affine + y2v run here on DVE: globp landed mid-layer, so
        # these clear the queue before AR_e returns.
        affine_p(0, CNT_P)
        y2v_scale()
        fillers(_NFILL, y1s[GPC - 1][:, 2 * L - DW:2 * L])

        # ================= LAYERS 2..3 ===================================
        for li in (1, 2):
            last = (li == 2)
            hs = [None] * (GPC + 1)
            if li == 1:
                # post-AR_e boundary: element affine + per-copy prescale;
                # y2v was precomputed at the end of layer 1.
                affine_e(0)
                prescale_g(0)
                phase3_g(li, 0, nchunks=2)
            else:
                # pooled affine + y2v interleaved with g0's phase3
                affine_p(1, CNT_P)
                cl = L // 2
                for ch in range(2):
                    for cot in range(2):
                        sl = slice(cot * L + ch * cl, cot * L + (ch + 1) * cl)
                        vec.tensor_scalar(poolb[:, sl], y1s[GPC][:, sl],
                                          a2[:, cot:cot + 1],
                                          bs[:, cot:cot + 1],
                                          OP.mult, OP.add)
                    for cot in range(2):
                        sl = slice(cot * L + ch * cl, cot * L + (ch + 1) * cl)
                        vec.tensor_tensor(y1s[0][:, sl], poolb[:, sl],
                                          y1s[0][:, sl], OP.add)
                        act.activation(y1s[0][:, sl], y1s[0][:, sl], AF.Relu)
            hs[0] = emit_h(li, 0, y1s[0])
            if not last:
                for g in range(1, GPC):
                    if g == 2:
                        # warm-up collective: re-sync cores mid-layer so
                        # the boundary AllReduces see minimal arrival skew
                        # (anchored on this layer's first accum slot).
                        sync.dma_start(warm2_in[:], s1e[:, 0:1])
                        _cc(nc, "AllReduce", OP.add, ALL8,
                            warm2_in[:], warm2_out[:])
                    if li == 1:
                        prescale_g(g)
                    phase3_g(li, g)
                    if g == 1:
                        vec.tensor_tensor(mxsb[:], y1s[0][:], y1s[1][:],
                                          OP.max)
                    else:
                        vec.tensor_tensor(mxsb[:], mxsb[:], y1s[g][:], OP.max)
                    emit_A(li, g - 1, hs[g - 1])
                    hs[g] = emit_h(li, g, y1s[g])
                    if g == GPC - 1:
                        # pool-max exchange feeding THIS layer's pooled inst
                        sync.dma_start(mx_in[:], mxsb[:])
                        _cc(nc, "AllReduce", OP.max, PAIRS,
                            mx_in[:], mx_out[:])
                        sync.dma_start(poolb[:], mx_out[:])
                emit_A(li, GPC - 1, hs[GPC - 1])
                pack_launch_e(1)
                # pooled instance (hides AR_e)
                hs[GPC] = emit_h(li, GPC, poolb)
                emit_A(li, GPC, hs[GPC])
                pack_launch_p(1)
                fillers(_NFILL, y1s[GPC][:, 2 * L - DW:2 * L])
                prescale(li)
            else:
                # L3 tensor order: h0 A0 h1 A1 h2 A2 hp Ap h3 A3; pre-BN
                # y1/y2 halves + stat sums stream out, host finishes BN+relu.
                phase3_g(li, 1)
                vec.tensor_tensor(mxsb[:], y1s[0][:], y1s[1][:], OP.max)
                emit_A(li, 0, hs[0], out_dram=out_d[0, :, :])
                hs[1] = emit_h(li, 1, y1s[1])
                phase3_g(li, 2)
                vec.tensor_tensor(mxsb[:], mxsb[:], y1s[2][:], OP.max)
                emit_A(li, 1, hs[1], out_dram=out_d[1, :, :])
                hs[2] = emit_h(li, 2, y1s[2])
                phase3_g(li, 3)
                vec.tensor_tensor(mxsb[:], mxsb[:], y1s[3][:], OP.max)
                sync.dma_start(mx_in[:], mxsb[:])
                _cc(nc, "AllReduce", OP.max, PAIRS, mx_in[:], mx_out[:])
                sync.dma_start(poolb[:], mx_out[:])
                emit_A(li, 2, hs[2], out_dram=out_d[2, :, :])
                hs[GPC] = emit_h(li, GPC, poolb)
                emit_A(li, GPC, hs[GPC], nq=2, asrc=Apsb,
                       out_dram=out2_d)
                for cot in range(2):
                    vec.reduce_sum(packp[:, cot:cot + 1],
                                   s1p[:, cot * 2:(cot + 1) * 2], axis=AX.X)
                    vec.reduce_sum(packp[:, 2 + cot:3 + cot],
                                   s2p[:, cot * 2:(cot + 1) * 2], axis=AX.X)
                sync.dma_start(outst_d[:, 4:8], packp[:])
                hs[3] = emit_h(li, 3, y1s[3])
                emit_A(li, 3, hs[3], out_dram=out_d[3, :, :])
                for cot in range(2):
                    vec.reduce_sum(packe[:, cot:cot + 1],
                                   s1e[:, cot * GPC * DQQ:
                                       (cot + 1) * GPC * DQQ], axis=AX.X)
                    vec.reduce_sum(packe[:, 2 + cot:3 + cot],
                                   s2e[:, cot * GPC * DQQ:
                                       (cot + 1) * GPC * DQQ], axis=AX.X)
                sync.dma_start(outst_d[:, 0:4], packe[:])


def _build():
    key = ("nc", _NO_CC)
    if key in _CACHE:
        return _CACHE[key]
    nc = bacc.Bacc("TRN2", target_bir_lowering=False, debug=False,
                   num_devices=NCORES)
    io = {
        "xsh": nc.dram_tensor("xsh", [GPC, 128, LT * 128], bf16,
                              kind="ExternalInput"),
        "pxsh": nc.dram_tensor("pxsh", [128, LT * 128], bf16,
                               kind="ExternalInput"),
        "Ash": nc.dram_tensor("Ash", [128, DQQ, LT * DW], bf16,
                              kind="ExternalInput"),
        "AshP": nc.dram_tensor("AshP", [128, 2, LT * DW], bf16,
                               kind="ExternalInput"),
        "Wmats": nc.dram_tensor("Wmats", [6, 128, 512], bf16,
                                kind="ExternalInput"),
        "gbs": nc.dram_tensor("gbs", [128, 24], f32, kind="ExternalInput"),
        "out": nc.dram_tensor("out", [GPC, 128, 2 * L], bf16,
                              kind="ExternalOutput"),
        "out2": nc.dram_tensor("out2", [128, 4 * DW], bf16,
                               kind="ExternalOutput"),
        "outst": nc.dram_tensor("outst", [128, 8], f32,
                                kind="ExternalOutput"),
    }
    with tile.TileContext(nc) as tc:
        _emit(tc, nc, io)
    nc.compile()
    _CACHE[key] = nc
    return nc


def _bf16(a):
    return np.asarray(a, np.float32).astype(ml_dtypes.bfloat16)


def _host_prep(edge_index, Ws, gs, bes):
    """Build the device-layout arrays on host."""
    src = np.asarray(edge_index[0], dtype=np.int64)
    dst = np.asarray(edge_index[1], dtype=np.int64)
    deg = np.zeros(L, np.float32)
    np.add.at(deg, dst, np.float32(1.0))
    deg += np.float32(2.0)
    dis = (1.0 / np.sqrt(deg.astype(np.float64))).astype(np.float32)
    A = np.zeros((L, L), np.float32)
    np.add.at(A, (src, dst), dis[src] * dis[dst])
    A[np.arange(L), np.arange(L)] += np.float32(2.0) * dis * dis
    ash = _bf16(np.ascontiguousarray(
        A.reshape(LT, 128, DQQ, DW).transpose(1, 2, 0, 3).reshape(128, DQQ, LT * DW)))

    wm = np.zeros((6, 128, 512), np.float32)
    for i, W in enumerate(Ws):
        cin = W.shape[0]
        wm[i, :, : (cin // 128) * 256] = np.ascontiguousarray(
            W.reshape(cin // 128, 128, 256).transpose(1, 0, 2).reshape(128, -1))
    wm = _bf16(wm)

    gb = np.zeros((128, 24), np.float32)
    vecs = [gs[0], bes[0], gs[1], bes[1], gs[2], bes[2],
            gs[3], bes[3], gs[4], bes[4], gs[5], bes[5]]
    for v, w in enumerate(vecs):
        gb[:, v * 2 + 0] = w[0:128]
        gb[:, v * 2 + 1] = w[128:256]
    return ash, wm, gb


def kernel(x, edge_index, W1, b1, W2, b2, W3, b3, W1s, b1s, W2s, b2s, W3s, b3s,
           g1, be1, g2, be2, g3, be3, g1s, be1s, g2s, be2s, g3s, be3s):
    x = np.asarray(x, np.float32)
    ash, wm, gb = _host_prep(
        np.asarray(edge_index),
        [np.asarray(W1, np.float32), np.asarray(W2, np.float32),
         np.asarray(W3, np.float32), np.asarray(W1s, np.float32),
         np.asarray(W2s, np.float32), np.asarray(W3s, np.float32)],
        [np.asarray(g1, np.float32), np.asarray(g2, np.float32),
         np.asarray(g3, np.float32), np.asarray(g1s, np.float32),
         np.asarray(g2s, np.float32), np.asarray(g3s, np.float32)],
        [np.asarray(be1, np.float32), np.asarray(be2, np.float32),
         np.asarray(be3, np.float32), np.asarray(be1s, np.float32),
         np.asarray(be2s, np.float32), np.asarray(be3s, np.float32)])

    # core k: graph b=k//2, copies n in [4*(k%2), 4*(k%2)+4)
    # upload x node-major: slot[p, st*128 + c] = x[st*128+p, c]
    xr = x.reshape(NCORES, GPC, CH[0], L)
    xnm = _bf16(np.ascontiguousarray(
        xr.reshape(NCORES, GPC, 128, LT, 128).transpose(0, 1, 4, 3, 2)
          .reshape(NCORES, GPC, 128, LT * 128)))
    # pooled-branch input: max over the 8 copies of each graph, node-major
    xp = x.reshape(B, N, CH[0], L).max(axis=1)  # [B, 128, L]
    xpnm = _bf16(np.ascontiguousarray(
        xp.reshape(B, 128, LT, 128).transpose(0, 3, 2, 1)
          .reshape(B, 128, LT * 128)))
    in_maps = []
    for k in range(NCORES):
        par = k % 2
        in_maps.append({
            "xsh": xnm[k], "pxsh": xpnm[k // 2],
            "Ash": ash, "AshP": np.ascontiguousarray(ash[:, 2 * par:2 * par + 2, :]),
            "Wmats": wm, "gbs": gb,
        })

    nc = _build()

    if _SIMULATE:
        from concourse.bass_interp import MultiCoreSim
        sim = MultiCoreSim(nc, NCORES)
        for k in range(NCORES):
            for nm, arr in in_maps[k].items():
                sim.cores[k].tensor(nm)[:] = arr
        sim.simulate(check_with_hw=False)
        outs = [np.array(sim.cores[k].mem_tensor("out")) for k in range(NCORES)]
        outs2 = [np.array(sim.cores[k].mem_tensor("out2")) for k in range(NCORES)]
        outsst = [np.array(sim.cores[k].mem_tensor("outst")) for k in range(NCORES)]
    else:
        res = run_bass_kernel_spmd(nc, in_maps, core_ids=list(range(NCORES)),
                                   trace=_PROFILE)
        if _PROFILE:
            _CACHE["last_result"] = res
        outs = [np.asarray(res.results[k]["out"]) for k in range(NCORES)]
        outs2 = [np.asarray(res.results[k]["out2"]) for k in range(NCORES)]
        outsst = [np.asarray(res.results[k]["outst"]) for k in range(NCORES)]

    # ---- host-side final layer: BN affine + pooled add + relu ----------
    # buf [GPC, 128, 2*L] bf16: y[g, cot*128+p, n] = buf[g, p, cot*L+n]
    y1 = np.stack([o.astype(np.float32) for o in outs])        # [8,G,128,2L]
    y1 = (y1.reshape(NCORES, GPC, 128, 2, L).transpose(0, 1, 3, 2, 4)
            .reshape(NCORES, GPC, 256, L))
    # out2 halves: core 2b+par holds dest-node cols (2*par+d2)*512+j
    y2 = np.empty((B, 256, L), np.float32)
    for b in range(B):
        for par in range(2):
            h = outs2[2 * b + par].astype(np.float32)  # [128, 4*DW]
            h = h.reshape(128, 2, 2, DW)               # [p, cot, d2, j]
            for cot in range(2):
                for d2 in range(2):
                    nd = (2 * par + d2) * DW
                    y2[b, cot * 128:(cot + 1) * 128, nd:nd + DW] = \
                        h[:, cot, d2, :]
    st = np.sum(np.stack([o.astype(np.float64) for o in outsst]), axis=0)

    def bn_affine(s1, s2, cnt, g, be):
        m = s1 / cnt
        v = s2 / cnt - m * m
        a = np.asarray(g, np.float64) / np.sqrt(v + EPS)
        return (a.astype(np.float32),
                (np.asarray(be, np.float64) - a * m).astype(np.float32))

    a1, b1 = bn_affine(st[:, 0:2].T.reshape(256), st[:, 2:4].T.reshape(256),
                       CNT_E, g3, be3)
    a2, b2 = bn_affine(st[:, 4:6].T.reshape(256), st[:, 6:8].T.reshape(256),
                       CNT_P3, g3s, be3s)
    bsum = (b1 + b2)[None, :, None]
    out = np.empty((NCORES * GPC, 256, L), np.float32)
    for k in range(NCORES):
        out[k * GPC:(k + 1) * GPC] = (a1[None, :, None] * y1[k]
                                      + a2[None, :, None] * y2[k // 2] + bsum)
    np.maximum(out, 0.0, out=out)
    return out


# revision 30
# speedup vs baseline: 1.0659x; 1.0242x over previous
"""Trainium2 Bass kernel for nn_DeepSymmetricGCN1dBlock.

3-layer GCN block over a shared 2048-node graph, 32 graph copies (b=4, n=8),
channels 128->256->256->256, per-element branch + symmetric max-pooled branch,
training-mode BatchNorm, ReLU.

Strategy (v5)
-------------
Data-parallel over the 32 graph copies: core k holds copies of graph b=k//2,
n in [4*(k%2), 4*(k%2)+4).  The sparse GCN aggregation is a dense matmul
against the normalized adjacency A_hat [2048, 2048], kept RESIDENT in SBUF
in bf16 (8 MiB), streamed in N=512 moving chunks.  All matmul operands are
bf16 (PSUM accumulation stays fp32); BN statistics are fp32.

Layer 1 runs aggregation-first (agg = x^T A at Cin=128 width; x is uploaded
pre-transposed to node-major by the host), dqq-OUTER across the 4 element
instances so compute starts as soon as the first A chunk lands.  Layers 2-3
run W-first (h = x W, then y = h^T A); h psum is drained in [128,512] pairs
alternating ACT/DVE so drains keep up with the matmul stream.

Per layer: 4 element instances first, then element-stats AllReduce (hidden
under the pooled instance), then pooled-stats AllReduce.  During the pooled
AR flight the element BN affine is pre-applied in place
(t = a1*y1 + (b1+b2), DVE), so post-AR work is just
x' = relu(a2*y2 + t) per copy.  Pool-max AllReduce runs in bf16 (exact) and
lands during the next layer's element matmuls.  Layer 3 ships pre-BN y1/y2
plus stat sums; its pooled instance is pair-split by destination-node halves
(per-core Ash_pool input selects the half) and the host stitches + applies
the final BN affine + relu.
"""

import sys

if "/opt/trn_rl_repo" not in sys.path:
    sys.path.insert(0, "/opt/trn_rl_repo")

import numpy as np
import ml_dtypes

import concourse.bass as bass
import concourse.bacc as bacc
import concourse.mybir as mybir
import concourse.tile as tile
from concourse.bass_utils import run_bass_kernel_spmd

f32 = mybir.dt.float32
bf16 = mybir.dt.bfloat16
AF = mybir.ActivationFunctionType
OP = mybir.AluOpType
AX = mybir.AxisListType

B, N, L, E = 4, 8, 2048, 16384
CH = [128, 256, 256, 256]
EPS = 1e-5
NCORES = 8
GPC = 4            # graph copies per core
LT = L // 128      # 16 node tiles
DQQ = 4            # A streamed in DQQ chunks of DW destination columns
DW = L // DQQ      # 512
CNT_E = 32 * L     # element-branch BN count (all 32 copies)
CNT_P = 8 * L      # pooled-branch BN count L1/L2 (4 graphs, pair-redundant)
CNT_P3 = 4 * L     # pooled L3: pair-split, each node counted once

PAIRS = [[0, 1], [2, 3], [4, 5], [6, 7]]
ALL8 = [list(range(NCORES))]

import os
_PROFILE = False
_SIMULATE = False
_NO_CC = os.environ.get("K_NO_CC", "0") == "1"
_NFILL = int(os.environ.get("K_NFILL", "70"))
_CACHE = {}


def _cc(nc, kind, op, groups, bi, bo):
    """bi/bo are APs into DRAM bounce tiles."""
    if _NO_CC:
        nc.sync.dma_start(bo, bi)
    else:
        nc.gpsimd.collective_compute(kind, op, replica_groups=groups,
                                     ins=[bi.opt()], outs=[bo.opt()])


def _emit(tc, nc, io):
    sync, vec, act, te = nc.sync, nc.vector, nc.scalar, nc.tensor

    from contextlib import ExitStack

    ctx = ExitStack()
    with ctx:
        sb = ctx.enter_context(tc.tile_pool(name="sb", bufs=1))
        sb_slot = ctx.enter_context(tc.tile_pool(name="slots", bufs=GPC + 1))
        sb_y1 = ctx.enter_context(tc.tile_pool(name="y1", bufs=GPC + 2))
        sb_h = ctx.enter_context(tc.tile_pool(name="h", bufs=2))
        sb_agg = ctx.enter_context(tc.tile_pool(name="agg", bufs=3))
        sb_w = ctx.enter_context(tc.tile_pool(name="w", bufs=6))
        sb_small = ctx.enter_context(tc.tile_pool(name="small", bufs=28))
        ps_x = ctx.enter_context(tc.tile_pool(name="psx", bufs=2, space="PSUM"))
        ps_w = ctx.enter_context(tc.tile_pool(name="psw", bufs=3, space="PSUM"))
        ps_f = ctx.enter_context(tc.tile_pool(name="psf", bufs=1, space="PSUM"))
        dram = ctx.enter_context(tc.tile_pool(name="dram", bufs=1, space="DRAM"))

        # ---- persistent SBUF tiles -------------------------------------
        Asb = sb.tile([128, DQQ * LT * DW], bf16, tag="Asb")
        Apsb = sb.tile([128, 2 * LT * DW], bf16, tag="Apsb")  # L3 pool half
        slots = [sb_slot.tile([128, LT * 128], bf16, tag="slot", name=f"slot{i}")
                 for i in range(GPC)]
        pool_nm = sb_slot.tile([128, LT * 128], bf16, tag="slot", name="pool_nm")
        y1s = [sb_y1.tile([128, 2 * L], bf16, tag="y1", name=f"y1_{i}")
               for i in range(GPC + 1)]
        poolb = sb_y1.tile([128, 2 * L], bf16, tag="y1", name="poolb")
        mxsb = sb.tile([128, 2 * L], bf16, tag="mxsb")
        sqscr = sb.tile([128, L], bf16, tag="sqscr")
        wsb = [sb_w.tile([128, 512], bf16, tag="w", name=f"w{i}") for i in range(6)]
        gbt = sb.tile([128, 24], f32, tag="gbt")
        s1e = sb.tile([128, 2 * GPC * DQQ], f32, tag="s1e")   # [cot][g][dqq]
        s1p = sb.tile([128, 2 * DQQ], f32, tag="s1p")         # [cot][dqq]
        s2e = sb.tile([128, 2 * GPC * DQQ], f32, tag="s2e")   # [cot][g][dqq]
        s2p = sb.tile([128, 2 * DQQ], f32, tag="s2p")         # [cot][dqq]
        packe = sb.tile([128, 4], f32, tag="packe")
        packp = sb.tile([128, 4], f32, tag="packp")
        globe = sb.tile([128, 4], f32, tag="globe")
        globp = sb.tile([128, 4], f32, tag="globp")

        fpsum = ps_f.tile([128, DW], f32, tag="fpsum")

        # ---- DRAM bounce tiles for collectives -------------------------
        mx_in = dram.tile([128, 2 * L], bf16, tag="mxi")
        mx_out = dram.tile([128, 2 * L], bf16, tag="mxo")
        ste_in = dram.tile([128, 4], f32, tag="stei")
        ste_outs = [dram.tile([128, 4], f32, tag="steo", name=f"ste_out{i}",
                              addr_space="Shared") for i in range(2)]
        warm_in = dram.tile([128, 1], f32, tag="warmi")
        warm_out = dram.tile([128, 1], f32, tag="warmo", addr_space="Shared")
        warm2_in = dram.tile([128, 1], f32, tag="warm2i")
        warm2_out = dram.tile([128, 1], f32, tag="warm2o", addr_space="Shared")
        stp_in = dram.tile([128, 4], f32, tag="stpi")
        stp_outs = [dram.tile([128, 4], f32, tag="stpo", name=f"stp_out{i}",
                              addr_space="Shared") for i in range(2)]

        xsh_d, px_d, ash_d, ashp_d, w_d, gb_d, out_d, out2_d, outst_d = (
            io["xsh"], io["pxsh"], io["Ash"], io["AshP"], io["Wmats"],
            io["gbs"], io["out"], io["out2"], io["outst"])

        # small affine tiles
        eps_t = sb_small.tile([128, 1], f32, tag="sm", name="eps")
        vec.memset(eps_t[:], EPS)
        t0 = sb_small.tile([128, 2], f32, tag="sm", name="t0")
        a1 = sb_small.tile([128, 2], f32, tag="sm", name="a1")
        b1 = sb_small.tile([128, 2], f32, tag="sm", name="b1")
        a2 = sb_small.tile([128, 2], f32, tag="sm", name="a2")
        a2h = sb_small.tile([128, 2], bf16, tag="sm", name="a2h")
        bs = sb_small.tile([128, 2], f32, tag="bs", name="bs")
        me = sb_small.tile([128, 2], f32, tag="sm", name="me")
        ve = sb_small.tile([128, 2], f32, tag="sm", name="ve")


        # ---- input loads: the L1 pooled instance runs first, so its
        # inputs (pool_nm + Ash chunk 0, in small pieces) lead the queue.
        # warm-up collective: the first CC op pays the ring-setup /
        # rendezvous cost (~10-25us).  Pay it immediately -- the input tile
        # is never written (garbage values), so nothing upstream gates it.
        _cc(nc, "AllReduce", OP.add, ALL8, warm_in[:], warm_out[:])
        for p in range(4):
            sync.dma_start(pool_nm[:, p * 512:(p + 1) * 512],
                           px_d[:, p * 512:(p + 1) * 512])
        for p in range(4):
            sync.dma_start(Asb[:, p * 2048:(p + 1) * 2048],
                           ash_d[:, 0, p * 2048:(p + 1) * 2048])
        sync.dma_start(wsb[3][:], w_d[3, :, :])
        sync.dma_start(slots[0][:], xsh_d[0, :, :])
        sync.dma_start(wsb[0][:], w_d[0, :, :])
        sync.dma_start(Asb[:, LT * DW:2 * LT * DW], ash_d[:, 1, :])
        for g in range(1, GPC):
            sync.dma_start(slots[g][:], xsh_d[g, :, :])
        for dqq in range(2, DQQ):
            sync.dma_start(Asb[:, dqq * LT * DW:(dqq + 1) * LT * DW],
                           ash_d[:, dqq, :])
        for i in [1, 4, 2, 5]:
            sync.dma_start(wsb[i][:], w_d[i, :, :])
        sync.dma_start(gbt[:], gb_d[:, :])
        sync.dma_start(Apsb[:, 0:LT * DW], ashp_d[:, 0, :])
        sync.dma_start(Apsb[:, LT * DW:2 * LT * DW], ashp_d[:, 1, :])

        def affine(a_t, b_t, s1_ap, s2_ap, inv_cnt, gslc, beslc):
            # a = g * rsqrt(var+eps); b = be - a*mean
            vec.tensor_scalar(me[:], s1_ap, inv_cnt, None, OP.mult)
            vec.tensor_scalar(ve[:], s2_ap, inv_cnt, None, OP.mult)
            vec.tensor_tensor(t0[:], me[:], me[:], OP.mult)
            vec.tensor_tensor(ve[:], ve[:], t0[:], OP.subtract)
            act.activation(t0[:], ve[:], AF.Sqrt, bias=eps_t[:])
            vec.reciprocal(t0[:], t0[:])
            vec.tensor_tensor(a_t[:], gslc, t0[:], OP.mult)
            vec.tensor_tensor(t0[:], a_t[:], me[:], OP.mult)
            vec.tensor_tensor(b_t[:], beslc, t0[:], OP.subtract)

        def sq_chunk(y_ap, s2_slot):
            """Sum of squares of one [128, DW] drained chunk on DVE."""
            vec.scalar_tensor_tensor(sqscr[:, 0:DW], y_ap, 1.0, y_ap,
                                     OP.mult, OP.mult, accum_out=s2_slot)

        def emit_A_block(src, dqq, drain_dve):
            """L1-style: agg[:, :] = x_chunk^T A[:, dqq block]; one psum."""
            pa = ps_x.tile([128, DW], f32, tag="psx")
            for st in range(LT):
                te.matmul(pa[:], src[:, st * 128:(st + 1) * 128],
                          Asb[:, (dqq * LT + st) * DW:(dqq * LT + st + 1) * DW],
                          start=(st == 0), stop=(st == LT - 1))
            agg = sb_agg.tile([128, DW], bf16, tag="agg")
            if drain_dve:
                vec.tensor_copy(agg[:], pa[:])
            else:
                act.activation(agg[:], pa[:], AF.Copy)
            return agg

        def emit_W_block(li, g, dqq, agg):
            """Project agg (Cin wide) to the two cot halves of y."""
            we = wsb[li] if g < GPC else wsb[3 + li]
            dsty = y1s[g] if g < GPC else y1s[GPC]
            s1 = s1e if g < GPC else s1p
            s2 = s2e if g < GPC else s2p
            for cot in range(2):
                pw = ps_w.tile([128, DW], f32, tag="psw")
                te.matmul(pw[:], we[:, cot * 128:(cot + 1) * 128], agg[:],
                          start=True, stop=True)
                idx = (cot * GPC + g) * DQQ + dqq if g < GPC \
                    else cot * DQQ + dqq
                yap = dsty[:, cot * L + dqq * DW:cot * L + (dqq + 1) * DW]
                act.activation(yap, pw[:], AF.Copy,
                               accum_out=s1[:, idx:idx + 1])
                sq_chunk(yap, s2[:, idx:idx + 1])

        def emit_h(li, g, src):
            """h = src W for L2/L3; paired [128,512] psum, alt ACT/DVE drain."""
            we = wsb[li] if g < GPC else wsb[3 + li]
            h = sb_h.tile([128, LT * 256], bf16, tag="h")
            for sp in range(LT // 2):
                ph = ps_x.tile([128, 512], f32, tag="psx")
                for sub in range(2):
                    st = sp * 2 + sub
                    for ct in range(2):
                        te.matmul(ph[:, sub * 256:(sub + 1) * 256],
                                  src[:, ct * L + st * 128:
                                      ct * L + st * 128 + 128],
                                  we[:, ct * 256:(ct + 1) * 256],
                                  start=(ct == 0), stop=(ct == 1))
                if sp % 2 == 0:
                    act.activation(h[:, sp * 512:(sp + 1) * 512], ph[:],
                                   AF.Copy)
                else:
                    vec.tensor_copy(h[:, sp * 512:(sp + 1) * 512], ph[:])
            return h

        def emit_A(li, g, h, nq=DQQ, asrc=None, out_dram=None):
            """y[cot, dqq] = h^T A for L2/L3 (dqq-major, st accumulation)."""
            if asrc is None:
                asrc = Asb
            dsty = y1s[g] if g < GPC else y1s[GPC]
            s1 = s1e if g < GPC else s1p
            s2 = s2e if g < GPC else s2p
            for cot in range(2):
                for dqq in range(nq):
                    pw = ps_w.tile([128, DW], f32, tag="psw")
                    for st in range(LT):
                        te.matmul(pw[:],
                                  h[:, st * 256 + cot * 128:
                                    st * 256 + cot * 128 + 128],
                                  asrc[:, (dqq * LT + st) * DW:
                                       (dqq * LT + st + 1) * DW],
                                  start=(st == 0), stop=(st == LT - 1))
                    if g < GPC:
                        idx = (cot * GPC + g) * DQQ + dqq
                        col = cot * L + dqq * DW
                    else:
                        idx = cot * nq + dqq
                        col = cot * nq * DW + dqq * DW
                    yap = dsty[:, col:col + DW]
                    act.activation(yap, pw[:], AF.Copy,
                                   accum_out=s1[:, idx:idx + 1])
                    sq_chunk(yap, s2[:, idx:idx + 1])
                    if out_dram is not None:
                        sync.dma_start(out_dram[:, col:col + DW], yap)

        def pack_launch_e(li):
            for cot in range(2):
                vec.reduce_sum(packe[:, cot:cot + 1],
                               s1e[:, cot * GPC * DQQ:(cot + 1) * GPC * DQQ],
                               axis=AX.X)
                vec.reduce_sum(packe[:, 2 + cot:3 + cot],
                               s2e[:, cot * GPC * DQQ:(cot + 1) * GPC * DQQ],
                               axis=AX.X)
            sync.dma_start(ste_in[:], packe[:])
            _cc(nc, "AllReduce", OP.add, ALL8, ste_in[:], ste_outs[li][:])
            sync.dma_start(globe[:], ste_outs[li][:])

        def pack_launch_p(li, nq=DQQ):
            for cot in range(2):
                vec.reduce_sum(packp[:, cot:cot + 1],
                               s1p[:, cot * nq:(cot + 1) * nq], axis=AX.X)
                vec.reduce_sum(packp[:, 2 + cot:3 + cot],
                               s2p[:, cot * nq:(cot + 1) * nq], axis=AX.X)
            sync.dma_start(stp_in[:], packp[:])
            _cc(nc, "AllReduce", OP.add, ALL8, stp_in[:], stp_outs[li][:])
            sync.dma_start(globp[:], stp_outs[li][:])

        def affine_e(li):
            affine(a1, b1, globe[:, 0:2], globe[:, 2:4], 1.0 / CNT_E,
                   gbt[:, 4 * li:4 * li + 2], gbt[:, 4 * li + 2:4 * li + 4])

        def affine_p(pl, cnt):
            # pl = pooled layer index (0-based); writes a2/bs
            affine(a2, bs, globp[:, 0:2], globp[:, 2:4], 1.0 / cnt,
                   gbt[:, 12 + 4 * pl:14 + 4 * pl],
                   gbt[:, 14 + 4 * pl:16 + 4 * pl])

        def prescale_g(g):
            # t = a1*y1 + b1 in place (element-BN pre-application)
            for cot in range(2):
                vec.tensor_scalar(
                    y1s[g][:, cot * L:(cot + 1) * L],
                    y1s[g][:, cot * L:(cot + 1) * L],
                    a1[:, cot:cot + 1], b1[:, cot:cot + 1],
                    OP.mult, OP.add)

        def prescale(li):
            affine_e(li)
            for g in range(GPC):
                prescale_g(g)

        def y2v_scale(nchunks=2):
            # v = a2*y2 + b2 into poolb (free at the boundary); phase3 then
            # only needs an add + relu per copy.
            cl = L // nchunks
            for ch in range(nchunks):
                for cot in range(2):
                    sl = slice(cot * L + ch * cl, cot * L + (ch + 1) * cl)
                    vec.tensor_scalar(poolb[:, sl], y1s[GPC][:, sl],
                                      a2[:, cot:cot + 1], bs[:, cot:cot + 1],
                                      OP.mult, OP.add)

        def phase3_g(li, g, nchunks=1):
            # x'_g = relu(t_g + v);  t = a1*y1+b1 (prescaled), v in poolb
            cl = L // nchunks
            for ch in range(nchunks):
                for cot in range(2):
                    sl = slice(cot * L + ch * cl, cot * L + (ch + 1) * cl)
                    vec.tensor_tensor(y1s[g][:, sl], poolb[:, sl],
                                      y1s[g][:, sl], OP.add)
                    act.activation(y1s[g][:, sl], y1s[g][:, sl], AF.Relu)

        def fillers(n, anchor):
            # junk matmuls that keep the PE HAM clock-gate warm across a
            # collective wait; never read back.  The rhs reads the batch's
            # anchor (the last-drained chunk before the boundary) so the
            # scheduler cannot hoist it away; the 2-column stationary keeps
            # the PE "busy" for HAM at ~1/64 the energy of a full matmul.
            for _ in range(n):
                te.matmul(fpsum[0:2, :], wsb[0][:, 0:2], anchor,
                          start=True, stop=True)

        # ================= LAYER 1 (agg-first, dqq-outer) ================
        # The pooled instance's dqq-blocks lead each element round: its
        # stats AllReduce launches ~3/4 through the layer and hides under
        # the element tail, and each Ash chunk gets a full round of
        # compute cover while the next one streams in.
        # block schedule: pooled dqq-blocks run in rounds 0,1,2,2 so the
        # pooled stats AR launches ~60% through the layer, well hidden.
        sched = []
        for dqq in range(DQQ):
            if dqq < 3:
                sched.append((GPC, dqq))
            sched += [(g, dqq) for g in range(GPC)]
            if dqq == 2:
                sched.insert(len(sched) - 2, (GPC, 3))
        pendW = None           # (g, dqq, agg) carried one block behind
        for g, dqq in sched:
            agg = emit_A_block(pool_nm if g == GPC else slots[g], dqq,
                               drain_dve=(g % 2 == 1))
            if pendW is not None:
                emit_W_block(0, pendW[0], pendW[1], pendW[2])
                if pendW[0] == GPC and pendW[1] == DQQ - 1:
                    # pooled instance complete: launch its stats AR now,
                    # hidden under the remaining element tail.
                    pack_launch_p(0)
            pendW = (g, dqq, agg)
        emit_W_block(0, pendW[0], pendW[1], pendW[2])
        pack_launch_e(0)
        # pooled affine + y2v run here on DVE: globp landed mid-layer, so
        # these clear the queue before AR_e returns.
        affine_p(0, CNT_P)
        y2v_scale()
        fillers(_NFILL, y1s[GPC - 1][:, 2 * L - DW:2 * L])

        # ================= LAYERS 2..3 ===================================
        for li in (1, 2):
            last = (li == 2)
            hs = [None] * (GPC + 1)
            # boundary head (same for both): element affine + per-copy
            # prescale + cheap add/relu; y2v was precomputed during the
            # previous layer's last element instance.
            affine_e(li - 1)
            prescale_g(0)
            phase3_g(li, 0, nchunks=2)
            hs[0] = emit_h(li, 0, y1s[0])
            if not last:
                prescale_g(1)
                phase3_g(li, 1)
                vec.tensor_tensor(mxsb[:], y1s[0][:], y1s[1][:], OP.max)
                emit_A(li, 0, hs[0])
                hs[1] = emit_h(li, 1, y1s[1])
                # warm-up collective: re-sync cores mid-layer so the
                # boundary AllReduces see minimal arrival skew.
                sync.dma_start(warm2_in[:], s1e[:, 0:1])
                _cc(nc, "AllReduce", OP.add, ALL8, warm2_in[:], warm2_out[:])
                prescale_g(2)
                phase3_g(li, 2)
                vec.tensor_tensor(mxsb[:], mxsb[:], y1s[2][:], OP.max)
                emit_A(li, 1, hs[1])
                hs[2] = emit_h(li, 2, y1s[2])
                prescale_g(3)
                phase3_g(li, 3)
                vec.tensor_tensor(mxsb[:], mxsb[:], y1s[3][:], OP.max)
                # pool-max exchange feeding THIS layer's pooled instance
                sync.dma_start(mx_in[:], mxsb[:])
                _cc(nc, "AllReduce", OP.max, PAIRS, mx_in[:], mx_out[:])
                sync.dma_start(poolb[:], mx_out[:])
                emit_A(li, 2, hs[2])
                # pooled instance 4th: its stats AR hides under e3
                hs[GPC] = emit_h(li, GPC, poolb)
                emit_A(li, GPC, hs[GPC])
                pack_launch_p(1)
                hs[3] = emit_h(li, 3, y1s[3])
                # pooled affine + y2v for the NEXT boundary run during e3
                affine_p(1, CNT_P)
                y2v_scale()
                emit_A(li, 3, hs[3])
                pack_launch_e(1)
                fillers(_NFILL, y1s[3][:, 2 * L - DW:2 * L])
            else:
                # L3 tensor order: h0 A0 h1 A1 h2 A2 hp Ap h3 A3; pre-BN
                # y1/y2 halves + stat sums stream out, host finishes BN+relu.
                prescale_g(1)
                phase3_g(li, 1)
                vec.tensor_tensor(mxsb[:], y1s[0][:], y1s[1][:], OP.max)
                emit_A(li, 0, hs[0], out_dram=out_d[0, :, :])
                hs[1] = emit_h(li, 1, y1s[1])
                prescale_g(2)
                phase3_g(li, 2)
                vec.tensor_tensor(mxsb[:], mxsb[:], y1s[2][:], OP.max)
                emit_A(li, 1, hs[1], out_dram=out_d[1, :, :])
                hs[2] = emit_h(li, 2, y1s[2])
                prescale_g(3)
                phase3_g(li, 3)
                vec.tensor_tensor(mxsb[:], mxsb[:], y1s[3][:], OP.max)
                sync.dma_start(mx_in[:], mxsb[:])
                _cc(nc, "AllReduce", OP.max, PAIRS, mx_in[:], mx_out[:])
                sync.dma_start(poolb[:], mx_out[:])
                emit_A(li, 2, hs[2], out_dram=out_d[2, :, :])
                hs[GPC] = emit_h(li, GPC, poolb)
                emit_A(li, GPC, hs[GPC], nq=2, asrc=Apsb,
                       out_dram=out2_d)
                for cot in range(2):
                    vec.reduce_sum(packp[:, cot:cot + 1],
                                   s1p[:, cot * 2:(cot + 1) * 2], axis=AX.X)
                    vec.reduce_sum(packp[:, 2 + cot:3 + cot],
                                   s2p[:, cot * 2:(cot + 1) * 2], axis=AX.X)
                sync.dma_start(outst_d[:, 4:8], packp[:])
                hs[3] = emit_h(li, 3, y1s[3])
                emit_A(li, 3, hs[3], out_dram=out_d[3, :, :])
                for cot in range(2):
                    vec.reduce_sum(packe[:, cot:cot + 1],
                                   s1e[:, cot * GPC * DQQ:
                                       (cot + 1) * GPC * DQQ], axis=AX.X)
                    vec.reduce_sum(packe[:, 2 + cot:3 + cot],
                                   s2e[:, cot * GPC * DQQ:
                                       (cot + 1) * GPC * DQQ], axis=AX.X)
                sync.dma_start(outst_d[:, 0:4], packe[:])


def _build():
    key = ("nc", _NO_CC)
    if key in _CACHE:
        return _CACHE[key]
    nc = bacc.Bacc("TRN2", target_bir_lowering=False, debug=False,
                   num_devices=NCORES)
    io = {
        "xsh": nc.dram_tensor("xsh", [GPC, 128, LT * 128], bf16,
                              kind="ExternalInput"),
        "pxsh": nc.dram_tensor("pxsh", [128, LT * 128], bf16,
                               kind="ExternalInput"),
        "Ash": nc.dram_tensor("Ash", [128, DQQ, LT * DW], bf16,
                              kind="ExternalInput"),
        "AshP": nc.dram_tensor("AshP", [128, 2, LT * DW], bf16,
                               kind="ExternalInput"),
        "Wmats": nc.dram_tensor("Wmats", [6, 128, 512], bf16,
                                kind="ExternalInput"),
        "gbs": nc.dram_tensor("gbs", [128, 24], f32, kind="ExternalInput"),
        "out": nc.dram_tensor("out", [GPC, 128, 2 * L], bf16,
                              kind="ExternalOutput"),
        "out2": nc.dram_tensor("out2", [128, 4 * DW], bf16,
                               kind="ExternalOutput"),
        "outst": nc.dram_tensor("outst", [128, 8], f32,
                                kind="ExternalOutput"),
    }
    with tile.TileContext(nc) as tc:
        _emit(tc, nc, io)
    nc.compile()
    _CACHE[key] = nc
    return nc


def _bf16(a):
    return np.asarray(a, np.float32).astype(ml_dtypes.bfloat16)


def _host_prep(edge_index, Ws, gs, bes):
    """Build the device-layout arrays on host."""
    src = np.asarray(edge_index[0], dtype=np.int64)
    dst = np.asarray(edge_index[1], dtype=np.int64)
    deg = np.zeros(L, np.float32)
    np.add.at(deg, dst, np.float32(1.0))
    deg += np.float32(2.0)
    dis = (1.0 / np.sqrt(deg.astype(np.float64))).astype(np.float32)
    A = np.zeros((L, L), np.float32)
    np.add.at(A, (src, dst), dis[src] * dis[dst])
    A[np.arange(L), np.arange(L)] += np.float32(2.0) * dis * dis
    ash = _bf16(np.ascontiguousarray(
        A.reshape(LT, 128, DQQ, DW).transpose(1, 2, 0, 3).reshape(128, DQQ, LT * DW)))

    wm = np.zeros((6, 128, 512), np.float32)
    for i, W in enumerate(Ws):
        cin = W.shape[0]
        wm[i, :, : (cin // 128) * 256] = np.ascontiguousarray(
            W.reshape(cin // 128, 128, 256).transpose(1, 0, 2).reshape(128, -1))
    wm = _bf16(wm)

    gb = np.zeros((128, 24), np.float32)
    vecs = [gs[0], bes[0], gs[1], bes[1], gs[2], bes[2],
            gs[3], bes[3], gs[4], bes[4], gs[5], bes[5]]
    for v, w in enumerate(vecs):
        gb[:, v * 2 + 0] = w[0:128]
        gb[:, v * 2 + 1] = w[128:256]
    return ash, wm, gb


def kernel(x, edge_index, W1, b1, W2, b2, W3, b3, W1s, b1s, W2s, b2s, W3s, b3s,
           g1, be1, g2, be2, g3, be3, g1s, be1s, g2s, be2s, g3s, be3s):
    x = np.asarray(x, np.float32)
    ash, wm, gb = _host_prep(
        np.asarray(edge_index),
        [np.asarray(W1, np.float32), np.asarray(W2, np.float32),
         np.asarray(W3, np.float32), np.asarray(W1s, np.float32),
         np.asarray(W2s, np.float32), np.asarray(W3s, np.float32)],
        [np.asarray(g1, np.float32), np.asarray(g2, np.float32),
         np.asarray(g3, np.float32), np.asarray(g1s, np.float32),
         np.asarray(g2s, np.float32), np.asarray(g3s, np.float32)],
        [np.asarray(be1, np.float32), np.asarray(be2, np.float32),
         np.asarray(be3, np.float32), np.asarray(be1s, np.float32),
         np.asarray(be2s, np.float32), np.asarray(be3s, np.float32)])

    # core k: graph b=k//2, copies n in [4*(k%2), 4*(k%2)+4)
    # upload x node-major: slot[p, st*128 + c] = x[st*128+p, c]
    xr = x.reshape(NCORES, GPC, CH[0], L)
    xnm = _bf16(np.ascontiguousarray(
        xr.reshape(NCORES, GPC, 128, LT, 128).transpose(0, 1, 4, 3, 2)
          .reshape(NCORES, GPC, 128, LT * 128)))
    # pooled-branch input: max over the 8 copies of each graph, node-major
    xp = x.reshape(B, N, CH[0], L).max(axis=1)  # [B, 128, L]
    xpnm = _bf16(np.ascontiguousarray(
        xp.reshape(B, 128, LT, 128).transpose(0, 3, 2, 1)
          .reshape(B, 128, LT * 128)))
    in_maps = []
    for k in range(NCORES):
        par = k % 2
        in_maps.append({
            "xsh": xnm[k], "pxsh": xpnm[k // 2],
            "Ash": ash, "AshP": np.ascontiguousarray(ash[:, 2 * par:2 * par + 2, :]),
            "Wmats": wm, "gbs": gb,
        })

    nc = _build()

    if _SIMULATE:
        from concourse.bass_interp import MultiCoreSim
        sim = MultiCoreSim(nc, NCORES)
        for k in range(NCORES):
            for nm, arr in in_maps[k].items():
                sim.cores[k].tensor(nm)[:] = arr
        sim.simulate(check_with_hw=False)
        outs = [np.array(sim.cores[k].mem_tensor("out")) for k in range(NCORES)]
        outs2 = [np.array(sim.cores[k].mem_tensor("out2")) for k in range(NCORES)]
        outsst = [np.array(sim.cores[k].mem_tensor("outst")) for k in range(NCORES)]
    else:
        res = run_bass_kernel_spmd(nc, in_maps, core_ids=list(range(NCORES)),
                                   trace=_PROFILE)
        if _PROFILE:
            _CACHE["last_result"] = res
        outs = [np.asarray(res.results[k]["out"]) for k in range(NCORES)]
        outs2 = [np.asarray(res.results[k]["out2"]) for k in range(NCORES)]
        outsst = [np.asarray(res.results[k]["outst"]) for k in range(NCORES)]

    # ---- host-side final layer: BN affine + pooled add + relu ----------
    # buf [GPC, 128, 2*L] bf16: y[g, cot*128+p, n] = buf[g, p, cot*L+n]
    y1 = np.stack([o.astype(np.float32) for o in outs])        # [8,G,128,2L]
    y1 = (y1.reshape(NCORES, GPC, 128, 2, L).transpose(0, 1, 3, 2, 4)
            .reshape(NCORES, GPC, 256, L))
    # out2 halves: core 2b+par holds dest-node cols (2*par+d2)*512+j
    y2 = np.empty((B, 256, L), np.float32)
    for b in range(B):
        for par in range(2):
            h = outs2[2 * b + par].astype(np.float32)  # [128, 4*DW]
            h = h.reshape(128, 2, 2, DW)               # [p, cot, d2, j]
            for cot in range(2):
                for d2 in range(2):
                    nd = (2 * par + d2) * DW
                    y2[b, cot * 128:(cot + 1) * 128, nd:nd + DW] = \
                        h[:, cot, d2, :]
    st = np.sum(np.stack([o.astype(np.float64) for o in outsst]), axis=0)

    def bn_affine(s1, s2, cnt, g, be):
        m = s1 / cnt
        v = s2 / cnt - m * m
        a = np.asarray(g, np.float64) / np.sqrt(v + EPS)
        return (a.astype(np.float32),
                (np.asarray(be, np.float64) - a * m).astype(np.float32))

    a1, b1 = bn_affine(st[:, 0:2].T.reshape(256), st[:, 2:4].T.reshape(256),
                       CNT_E, g3, be3)
    a2, b2 = bn_affine(st[:, 4:6].T.reshape(256), st[:, 6:8].T.reshape(256),
                       CNT_P3, g3s, be3s)
    bsum = (b1 + b2)[None, :, None]
    out = np.empty((NCORES * GPC, 256, L), np.float32)
    for k in range(NCORES):
        out[k * GPC:(k + 1) * GPC] = (a1[None, :, None] * y1[k]
                                      + a2[None, :, None] * y2[k // 2] + bsum)
    np.maximum(out, 0.0, out=out)
    return out


# revision 31
# speedup vs baseline: 1.0689x; 1.0029x over previous
"""Trainium2 Bass kernel for nn_DeepSymmetricGCN1dBlock.

3-layer GCN block over a shared 2048-node graph, 32 graph copies (b=4, n=8),
channels 128->256->256->256, per-element branch + symmetric max-pooled branch,
training-mode BatchNorm, ReLU.

Strategy (v6)
-------------
Data-parallel over the 32 graph copies: core k holds copies of graph b=k//2,
n in [4*(k%2), 4*(k%2)+4).  The sparse GCN aggregation is a dense matmul
against the normalized adjacency A_hat [2048, 2048], kept RESIDENT in SBUF
in bf16 (8 MiB), streamed in N=512 moving chunks.  All matmul operands are
bf16 (PSUM accumulation stays fp32); BN statistics are fp32.

Layer 1 runs aggregation-first (agg = x^T A at Cin=128 width; x is uploaded
pre-transposed to node-major, in small leading pieces so compute starts
~8us in), dqq-OUTER with the pooled instance's blocks spread over rounds
0-2 so its stats AllReduce launches ~60% through the layer, fully hidden.
Layers 2-3 run W-first (h = x W, then y = h^T A); h psum is drained in
[128,512] pairs alternating ACT/DVE.

Collective discipline: a dependency-free warm-up AllReduce pays the ~15us
first-op rendezvous cost at kernel start; per layer the pooled instance
runs 4th so its stats AR hides under the last element instance, leaving
only the element-stats AR (~9us) exposed at each boundary.  Sum/sum-sq BN
stats accumulate per drained [128,512] chunk (ACT accum_out + DVE square),
so the AR trigger path is ~2us.  During the final element instance the
pooled BN affine is pre-applied (v = a2*y2 + b2 into the free poolb tile);
after the AR the boundary costs only affine_e + t = a1*y1+b1 (copy 0) +
relu(t + v).  Low-power filler matmuls (2-column stationary) anchored to
the last pre-boundary drain keep the PE HAM clock-gate warm across each
AR wait.  The pool-max AllReduce runs in bf16 (exact) over core pairs and
lands during the next layer's element matmuls.  Layer 3 ships pre-BN
y1/y2 + stat sums per chunk as they drain; its pooled instance is
pair-split by destination-node halves (per-core Ash_pool input selects
the half) and the host stitches + applies the final BN affine + relu.
"""

import sys

if "/opt/trn_rl_repo" not in sys.path:
    sys.path.insert(0, "/opt/trn_rl_repo")

import numpy as np
import ml_dtypes

import concourse.bass as bass
import concourse.bacc as bacc
import concourse.mybir as mybir
import concourse.tile as tile
from concourse.bass_utils import run_bass_kernel_spmd

f32 = mybir.dt.float32
bf16 = mybir.dt.bfloat16
AF = mybir.ActivationFunctionType
OP = mybir.AluOpType
AX = mybir.AxisListType

B, N, L, E = 4, 8, 2048, 16384
CH = [128, 256, 256, 256]
EPS = 1e-5
NCORES = 8
GPC = 4            # graph copies per core
LT = L // 128      # 16 node tiles
DQQ = 4            # A streamed in DQQ chunks of DW destination columns
DW = L // DQQ      # 512
CNT_E = 32 * L     # element-branch BN count (all 32 copies)
CNT_P = 8 * L      # pooled-branch BN count L1/L2 (4 graphs, pair-redundant)
CNT_P3 = 4 * L     # pooled L3: pair-split, each node counted once

PAIRS = [[0, 1], [2, 3], [4, 5], [6, 7]]
ALL8 = [list(range(NCORES))]

import os
_PROFILE = False
_SIMULATE = False
_NO_CC = os.environ.get("K_NO_CC", "0") == "1"
_NFILL = int(os.environ.get("K_NFILL", "70"))
_CACHE = {}


def _cc(nc, kind, op, groups, bi, bo):
    """bi/bo are APs into DRAM bounce tiles."""
    if _NO_CC:
        nc.sync.dma_start(bo, bi)
    else:
        nc.gpsimd.collective_compute(kind, op, replica_groups=groups,
                                     ins=[bi.opt()], outs=[bo.opt()])


def _emit(tc, nc, io):
    sync, vec, act, te = nc.sync, nc.vector, nc.scalar, nc.tensor

    from contextlib import ExitStack

    ctx = ExitStack()
    with ctx:
        sb = ctx.enter_context(tc.tile_pool(name="sb", bufs=1))
        sb_slot = ctx.enter_context(tc.tile_pool(name="slots", bufs=GPC + 1))
        sb_y1 = ctx.enter_context(tc.tile_pool(name="y1", bufs=GPC + 2))
        sb_h = ctx.enter_context(tc.tile_pool(name="h", bufs=2))
        sb_agg = ctx.enter_context(tc.tile_pool(name="agg", bufs=3))
        sb_w = ctx.enter_context(tc.tile_pool(name="w", bufs=6))
        sb_small = ctx.enter_context(tc.tile_pool(name="small", bufs=28))
        ps_x = ctx.enter_context(tc.tile_pool(name="psx", bufs=2, space="PSUM"))
        ps_w = ctx.enter_context(tc.tile_pool(name="psw", bufs=3, space="PSUM"))
        ps_f = ctx.enter_context(tc.tile_pool(name="psf", bufs=1, space="PSUM"))
        dram = ctx.enter_context(tc.tile_pool(name="dram", bufs=1, space="DRAM"))

        # ---- persistent SBUF tiles -------------------------------------
        Asb = sb.tile([128, DQQ * LT * DW], bf16, tag="Asb")
        Apsb = sb.tile([128, 2 * LT * DW], bf16, tag="Apsb")  # L3 pool half
        slots = [sb_slot.tile([128, LT * 128], bf16, tag="slot", name=f"slot{i}")
                 for i in range(GPC)]
        pool_nm = sb_slot.tile([128, LT * 128], bf16, tag="slot", name="pool_nm")
        y1s = [sb_y1.tile([128, 2 * L], bf16, tag="y1", name=f"y1_{i}")
               for i in range(GPC + 1)]
        poolb = sb_y1.tile([128, 2 * L], bf16, tag="y1", name="poolb")
        mxsb = sb.tile([128, 2 * L], bf16, tag="mxsb")
        sqscr = sb.tile([128, L], bf16, tag="sqscr")
        wsb = [sb_w.tile([128, 512], bf16, tag="w", name=f"w{i}") for i in range(6)]
        gbt = sb.tile([128, 24], f32, tag="gbt")
        s1e = sb.tile([128, 2 * GPC * DQQ], f32, tag="s1e")   # [cot][g][dqq]
        s1p = sb.tile([128, 2 * DQQ], f32, tag="s1p")         # [cot][dqq]
        s2e = sb.tile([128, 2 * GPC * DQQ], f32, tag="s2e")   # [cot][g][dqq]
        s2p = sb.tile([128, 2 * DQQ], f32, tag="s2p")         # [cot][dqq]
        packe = sb.tile([128, 4], f32, tag="packe")
        packp = sb.tile([128, 4], f32, tag="packp")
        globe = sb.tile([128, 4], f32, tag="globe")
        globp = sb.tile([128, 4], f32, tag="globp")

        fpsum = ps_f.tile([128, DW], f32, tag="fpsum")

        # ---- DRAM bounce tiles for collectives -------------------------
        mx_in = dram.tile([128, 2 * L], bf16, tag="mxi")
        mx_out = dram.tile([128, 2 * L], bf16, tag="mxo")
        ste_in = dram.tile([128, 4], f32, tag="stei")
        ste_outs = [dram.tile([128, 4], f32, tag="steo", name=f"ste_out{i}",
                              addr_space="Shared") for i in range(2)]
        warm_in = dram.tile([128, 1], f32, tag="warmi")
        warm_out = dram.tile([128, 1], f32, tag="warmo", addr_space="Shared")
        warm2_in = dram.tile([128, 1], f32, tag="warm2i")
        warm2_out = dram.tile([128, 1], f32, tag="warm2o", addr_space="Shared")
        stp_in = dram.tile([128, 4], f32, tag="stpi")
        stp_outs = [dram.tile([128, 4], f32, tag="stpo", name=f"stp_out{i}",
                              addr_space="Shared") for i in range(2)]

        xsh_d, px_d, ash_d, ashp_d, w_d, gb_d, out_d, out2_d, outst_d = (
            io["xsh"], io["pxsh"], io["Ash"], io["AshP"], io["Wmats"],
            io["gbs"], io["out"], io["out2"], io["outst"])

        # small affine tiles
        eps_t = sb_small.tile([128, 1], f32, tag="sm", name="eps")
        vec.memset(eps_t[:], EPS)
        t0 = sb_small.tile([128, 2], f32, tag="sm", name="t0")
        a1 = sb_small.tile([128, 2], f32, tag="sm", name="a1")
        b1 = sb_small.tile([128, 2], f32, tag="sm", name="b1")
        a2 = sb_small.tile([128, 2], f32, tag="sm", name="a2")
        a2h = sb_small.tile([128, 2], bf16, tag="sm", name="a2h")
        bs = sb_small.tile([128, 2], f32, tag="bs", name="bs")
        me = sb_small.tile([128, 2], f32, tag="sm", name="me")
        ve = sb_small.tile([128, 2], f32, tag="sm", name="ve")


        # ---- input loads: the L1 pooled instance runs first, so its
        # inputs (pool_nm + Ash chunk 0, in small pieces) lead the queue.
        # warm-up collective: the first CC op pays the ring-setup /
        # rendezvous cost (~10-25us).  Pay it immediately -- the input tile
        # is never written (garbage values), so nothing upstream gates it.
        _cc(nc, "AllReduce", OP.add, ALL8, warm_in[:], warm_out[:])
        for p in range(4):
            sync.dma_start(pool_nm[:, p * 512:(p + 1) * 512],
                           px_d[:, p * 512:(p + 1) * 512])
        for p in range(4):
            sync.dma_start(Asb[:, p * 2048:(p + 1) * 2048],
                           ash_d[:, 0, p * 2048:(p + 1) * 2048])
        sync.dma_start(wsb[3][:], w_d[3, :, :])
        sync.dma_start(slots[0][:], xsh_d[0, :, :])
        sync.dma_start(wsb[0][:], w_d[0, :, :])
        sync.dma_start(Asb[:, LT * DW:2 * LT * DW], ash_d[:, 1, :])
        for g in range(1, GPC):
            sync.dma_start(slots[g][:], xsh_d[g, :, :])
        for dqq in range(2, DQQ):
            sync.dma_start(Asb[:, dqq * LT * DW:(dqq + 1) * LT * DW],
                           ash_d[:, dqq, :])
        for i in [1, 4, 2, 5]:
            sync.dma_start(wsb[i][:], w_d[i, :, :])
        sync.dma_start(gbt[:], gb_d[:, :])
        sync.dma_start(Apsb[:, 0:LT * DW], ashp_d[:, 0, :])
        sync.dma_start(Apsb[:, LT * DW:2 * LT * DW], ashp_d[:, 1, :])

        def affine(a_t, b_t, s1_ap, s2_ap, inv_cnt, gslc, beslc):
            # a = g * rsqrt(var+eps); b = be - a*mean
            vec.tensor_scalar(me[:], s1_ap, inv_cnt, None, OP.mult)
            vec.tensor_scalar(ve[:], s2_ap, inv_cnt, None, OP.mult)
            vec.tensor_tensor(t0[:], me[:], me[:], OP.mult)
            vec.tensor_tensor(ve[:], ve[:], t0[:], OP.subtract)
            act.activation(t0[:], ve[:], AF.Sqrt, bias=eps_t[:])
            vec.reciprocal(t0[:], t0[:])
            vec.tensor_tensor(a_t[:], gslc, t0[:], OP.mult)
            vec.tensor_tensor(t0[:], a_t[:], me[:], OP.mult)
            vec.tensor_tensor(b_t[:], beslc, t0[:], OP.subtract)

        def sq_chunk(y_ap, s2_slot):
            """Sum of squares of one [128, DW] drained chunk on DVE."""
            vec.scalar_tensor_tensor(sqscr[:, 0:DW], y_ap, 1.0, y_ap,
                                     OP.mult, OP.mult, accum_out=s2_slot)

        def emit_A_block(src, dqq, drain_dve):
            """L1-style: agg[:, :] = x_chunk^T A[:, dqq block]; one psum."""
            pa = ps_x.tile([128, DW], f32, tag="psx")
            for st in range(LT):
                te.matmul(pa[:], src[:, st * 128:(st + 1) * 128],
                          Asb[:, (dqq * LT + st) * DW:(dqq * LT + st + 1) * DW],
                          start=(st == 0), stop=(st == LT - 1))
            agg = sb_agg.tile([128, DW], bf16, tag="agg")
            if drain_dve:
                vec.tensor_copy(agg[:], pa[:])
            else:
                act.activation(agg[:], pa[:], AF.Copy)
            return agg

        def emit_W_block(li, g, dqq, agg):
            """Project agg (Cin wide) to the two cot halves of y."""
            we = wsb[li] if g < GPC else wsb[3 + li]
            dsty = y1s[g] if g < GPC else y1s[GPC]
            s1 = s1e if g < GPC else s1p
            s2 = s2e if g < GPC else s2p
            for cot in range(2):
                pw = ps_w.tile([128, DW], f32, tag="psw")
                te.matmul(pw[:], we[:, cot * 128:(cot + 1) * 128], agg[:],
                          start=True, stop=True)
                idx = (cot * GPC + g) * DQQ + dqq if g < GPC \
                    else cot * DQQ + dqq
                yap = dsty[:, cot * L + dqq * DW:cot * L + (dqq + 1) * DW]
                act.activation(yap, pw[:], AF.Copy,
                               accum_out=s1[:, idx:idx + 1])
                sq_chunk(yap, s2[:, idx:idx + 1])

        def emit_h(li, g, src):
            """h = src W for L2/L3; paired [128,512] psum, alt ACT/DVE drain."""
            we = wsb[li] if g < GPC else wsb[3 + li]
            h = sb_h.tile([128, LT * 256], bf16, tag="h")
            for sp in range(LT // 2):
                ph = ps_x.tile([128, 512], f32, tag="psx")
                for sub in range(2):
                    st = sp * 2 + sub
                    for ct in range(2):
                        te.matmul(ph[:, sub * 256:(sub + 1) * 256],
                                  src[:, ct * L + st * 128:
                                      ct * L + st * 128 + 128],
                                  we[:, ct * 256:(ct + 1) * 256],
                                  start=(ct == 0), stop=(ct == 1))
                if sp % 2 == 0:
                    act.activation(h[:, sp * 512:(sp + 1) * 512], ph[:],
                                   AF.Copy)
                else:
                    vec.tensor_copy(h[:, sp * 512:(sp + 1) * 512], ph[:])
            return h

        def emit_A(li, g, h, nq=DQQ, asrc=None, out_dram=None):
            """y[cot, dqq] = h^T A for L2/L3 (dqq-major, st accumulation)."""
            if asrc is None:
                asrc = Asb
            dsty = y1s[g] if g < GPC else y1s[GPC]
            s1 = s1e if g < GPC else s1p
            s2 = s2e if g < GPC else s2p
            for cot in range(2):
                for dqq in range(nq):
                    pw = ps_w.tile([128, DW], f32, tag="psw")
                    for st in range(LT):
                        te.matmul(pw[:],
                                  h[:, st * 256 + cot * 128:
                                    st * 256 + cot * 128 + 128],
                                  asrc[:, (dqq * LT + st) * DW:
                                       (dqq * LT + st + 1) * DW],
                                  start=(st == 0), stop=(st == LT - 1))
                    if g < GPC:
                        idx = (cot * GPC + g) * DQQ + dqq
                        col = cot * L + dqq * DW
                    else:
                        idx = cot * nq + dqq
                        col = cot * nq * DW + dqq * DW
                    yap = dsty[:, col:col + DW]
                    act.activation(yap, pw[:], AF.Copy,
                                   accum_out=s1[:, idx:idx + 1])
                    sq_chunk(yap, s2[:, idx:idx + 1])
                    if out_dram is not None:
                        sync.dma_start(out_dram[:, col:col + DW], yap)

        def pack_launch_e(li):
            for cot in range(2):
                vec.reduce_sum(packe[:, cot:cot + 1],
                               s1e[:, cot * GPC * DQQ:(cot + 1) * GPC * DQQ],
                               axis=AX.X)
                vec.reduce_sum(packe[:, 2 + cot:3 + cot],
                               s2e[:, cot * GPC * DQQ:(cot + 1) * GPC * DQQ],
                               axis=AX.X)
            sync.dma_start(ste_in[:], packe[:])
            _cc(nc, "AllReduce", OP.add, ALL8, ste_in[:], ste_outs[li][:])
            sync.dma_start(globe[:], ste_outs[li][:])

        def pack_launch_p(li, nq=DQQ):
            for cot in range(2):
                vec.reduce_sum(packp[:, cot:cot + 1],
                               s1p[:, cot * nq:(cot + 1) * nq], axis=AX.X)
                vec.reduce_sum(packp[:, 2 + cot:3 + cot],
                               s2p[:, cot * nq:(cot + 1) * nq], axis=AX.X)
            sync.dma_start(stp_in[:], packp[:])
            _cc(nc, "AllReduce", OP.add, ALL8, stp_in[:], stp_outs[li][:])
            sync.dma_start(globp[:], stp_outs[li][:])

        def affine_e(li):
            affine(a1, b1, globe[:, 0:2], globe[:, 2:4], 1.0 / CNT_E,
                   gbt[:, 4 * li:4 * li + 2], gbt[:, 4 * li + 2:4 * li + 4])

        def affine_p(pl, cnt):
            # pl = pooled layer index (0-based); writes a2/bs
            affine(a2, bs, globp[:, 0:2], globp[:, 2:4], 1.0 / cnt,
                   gbt[:, 12 + 4 * pl:14 + 4 * pl],
                   gbt[:, 14 + 4 * pl:16 + 4 * pl])

        def prescale_g(g):
            # t = a1*y1 + b1 in place (element-BN pre-application)
            for cot in range(2):
                vec.tensor_scalar(
                    y1s[g][:, cot * L:(cot + 1) * L],
                    y1s[g][:, cot * L:(cot + 1) * L],
                    a1[:, cot:cot + 1], b1[:, cot:cot + 1],
                    OP.mult, OP.add)

        def prescale(li):
            affine_e(li)
            for g in range(GPC):
                prescale_g(g)

        def y2v_scale(nchunks=2):
            # v = a2*y2 + b2 into poolb (free at the boundary); phase3 then
            # only needs an add + relu per copy.
            cl = L // nchunks
            for ch in range(nchunks):
                for cot in range(2):
                    sl = slice(cot * L + ch * cl, cot * L + (ch + 1) * cl)
                    vec.tensor_scalar(poolb[:, sl], y1s[GPC][:, sl],
                                      a2[:, cot:cot + 1], bs[:, cot:cot + 1],
                                      OP.mult, OP.add)

        def phase3_g(li, g, nchunks=1):
            # x'_g = relu(t_g + v);  t = a1*y1+b1 (prescaled), v in poolb
            cl = L // nchunks
            for ch in range(nchunks):
                for cot in range(2):
                    sl = slice(cot * L + ch * cl, cot * L + (ch + 1) * cl)
                    vec.tensor_tensor(y1s[g][:, sl], poolb[:, sl],
                                      y1s[g][:, sl], OP.add)
                    act.activation(y1s[g][:, sl], y1s[g][:, sl], AF.Relu)

        def fillers(n, anchor):
            # junk matmuls that keep the PE HAM clock-gate warm across a
            # collective wait; never read back.  The rhs reads the batch's
            # anchor (the last-drained chunk before the boundary) so the
            # scheduler cannot hoist it away; the 2-column stationary keeps
            # the PE "busy" for HAM at ~1/64 the energy of a full matmul.
            for _ in range(n):
                te.matmul(fpsum[0:2, :], wsb[0][:, 0:2], anchor,
                          start=True, stop=True)

        # ================= LAYER 1 (agg-first, dqq-outer) ================
        # The pooled instance's dqq-blocks lead each element round: its
        # stats AllReduce launches ~3/4 through the layer and hides under
        # the element tail, and each Ash chunk gets a full round of
        # compute cover while the next one streams in.
        # block schedule: pooled dqq-blocks run in rounds 0,1,2,2 so the
        # pooled stats AR launches ~60% through the layer, well hidden.
        sched = []
        for dqq in range(DQQ):
            if dqq < 3:
                sched.append((GPC, dqq))
            sched += [(g, dqq) for g in range(GPC)]
            if dqq == 2:
                sched.insert(len(sched) - 2, (GPC, 3))
        pendW = None           # (g, dqq, agg) carried one block behind
        for g, dqq in sched:
            agg = emit_A_block(pool_nm if g == GPC else slots[g], dqq,
                               drain_dve=(g % 2 == 1))
            if pendW is not None:
                emit_W_block(0, pendW[0], pendW[1], pendW[2])
                if pendW[0] == GPC and pendW[1] == DQQ - 1:
                    # pooled instance complete: launch its stats AR now,
                    # hidden under the remaining element tail.
                    pack_launch_p(0)
            pendW = (g, dqq, agg)
        emit_W_block(0, pendW[0], pendW[1], pendW[2])
        pack_launch_e(0)
        # pooled affine + y2v run here on DVE: globp landed mid-layer, so
        # these clear the queue before AR_e returns.
        affine_p(0, CNT_P)
        y2v_scale()
        fillers(_NFILL, y1s[GPC - 1][:, 2 * L - DW:2 * L])

        # ================= LAYERS 2..3 ===================================
        for li in (1, 2):
            last = (li == 2)
            hs = [None] * (GPC + 1)
            # boundary head (same for both): element affine + per-copy
            # prescale + cheap add/relu; y2v was precomputed during the
            # previous layer's last element instance.
            affine_e(li - 1)
            prescale_g(0)
            phase3_g(li, 0, nchunks=2)
            hs[0] = emit_h(li, 0, y1s[0])
            if not last:
                prescale_g(1)
                phase3_g(li, 1)
                vec.tensor_tensor(mxsb[:], y1s[0][:], y1s[1][:], OP.max)
                emit_A(li, 0, hs[0])
                hs[1] = emit_h(li, 1, y1s[1])
                # warm-up collective: re-sync cores mid-layer so the
                # boundary AllReduces see minimal arrival skew.
                sync.dma_start(warm2_in[:], s1e[:, 0:1])
                _cc(nc, "AllReduce", OP.add, ALL8, warm2_in[:], warm2_out[:])
                prescale_g(2)
                phase3_g(li, 2)
                vec.tensor_tensor(mxsb[:], mxsb[:], y1s[2][:], OP.max)
                emit_A(li, 1, hs[1])
                hs[2] = emit_h(li, 2, y1s[2])
                prescale_g(3)
                phase3_g(li, 3)
                vec.tensor_tensor(mxsb[:], mxsb[:], y1s[3][:], OP.max)
                # pool-max exchange feeding THIS layer's pooled instance
                sync.dma_start(mx_in[:], mxsb[:])
                _cc(nc, "AllReduce", OP.max, PAIRS, mx_in[:], mx_out[:])
                sync.dma_start(poolb[:], mx_out[:])
                emit_A(li, 2, hs[2])
                # pooled instance 4th: its stats AR hides under e3
                hs[GPC] = emit_h(li, GPC, poolb)
                emit_A(li, GPC, hs[GPC])
                pack_launch_p(1)
                hs[3] = emit_h(li, 3, y1s[3])
                # pooled affine + y2v for the NEXT boundary run during e3
                affine_p(1, CNT_P)
                y2v_scale()
                emit_A(li, 3, hs[3])
                pack_launch_e(1)
                fillers(_NFILL, y1s[3][:, 2 * L - DW:2 * L])
            else:
                # L3 tensor order: h0 A0 h1 A1 h2 A2 hp Ap h3 A3; pre-BN
                # y1/y2 halves + stat sums stream out, host finishes BN+relu.
                prescale_g(1)
                phase3_g(li, 1)
                vec.tensor_tensor(mxsb[:], y1s[0][:], y1s[1][:], OP.max)
                emit_A(li, 0, hs[0], out_dram=out_d[0, :, :])
                hs[1] = emit_h(li, 1, y1s[1])
                prescale_g(2)
                phase3_g(li, 2)
                vec.tensor_tensor(mxsb[:], mxsb[:], y1s[2][:], OP.max)
                emit_A(li, 1, hs[1], out_dram=out_d[1, :, :])
                hs[2] = emit_h(li, 2, y1s[2])
                prescale_g(3)
                phase3_g(li, 3)
                vec.tensor_tensor(mxsb[:], mxsb[:], y1s[3][:], OP.max)
                sync.dma_start(mx_in[:], mxsb[:])
                _cc(nc, "AllReduce", OP.max, PAIRS, mx_in[:], mx_out[:])
                sync.dma_start(poolb[:], mx_out[:])
                emit_A(li, 2, hs[2], out_dram=out_d[2, :, :])
                hs[GPC] = emit_h(li, GPC, poolb)
                emit_A(li, GPC, hs[GPC], nq=2, asrc=Apsb,
                       out_dram=out2_d)
                for cot in range(2):
                    vec.reduce_sum(packp[:, cot:cot + 1],
                                   s1p[:, cot * 2:(cot + 1) * 2], axis=AX.X)
                    vec.reduce_sum(packp[:, 2 + cot:3 + cot],
                                   s2p[:, cot * 2:(cot + 1) * 2], axis=AX.X)
                sync.dma_start(outst_d[:, 4:8], packp[:])
                hs[3] = emit_h(li, 3, y1s[3])
                emit_A(li, 3, hs[3], out_dram=out_d[3, :, :])
                for cot in range(2):
                    vec.reduce_sum(packe[:, cot:cot + 1],
                                   s1e[:, cot * GPC * DQQ:
                                       (cot + 1) * GPC * DQQ], axis=AX.X)
                    vec.reduce_sum(packe[:, 2 + cot:3 + cot],
                                   s2e[:, cot * GPC * DQQ:
                                       (cot + 1) * GPC * DQQ], axis=AX.X)
                sync.dma_start(outst_d[:, 0:4], packe[:])


def _build():
    key = ("nc", _NO_CC)
    if key in _CACHE:
        return _CACHE[key]
    nc = bacc.Bacc("TRN2", target_bir_lowering=False, debug=False,
                   num_devices=NCORES)
    io = {
        "xsh": nc.dram_tensor("xsh", [GPC, 128, LT * 128], bf16,
                              kind="ExternalInput"),
        "pxsh": nc.dram_tensor("pxsh", [128, LT * 128], bf16,
                               kind="ExternalInput"),
        "Ash": nc.dram_tensor("Ash", [128, DQQ, LT * DW], bf16,
                              kind="ExternalInput"),
        "AshP": nc.dram_tensor("AshP", [128, 2, LT * DW], bf16,
                               kind="ExternalInput"),
        "Wmats": nc.dram_tensor("Wmats", [6, 128, 512], bf16,
                                kind="ExternalInput"),
        "gbs": nc.dram_tensor("gbs", [128, 24], f32, kind="ExternalInput"),
        "out": nc.dram_tensor("out", [GPC, 128, 2 * L], bf16,
                              kind="ExternalOutput"),
        "out2": nc.dram_tensor("out2", [128, 4 * DW], bf16,
                               kind="ExternalOutput"),
        "outst": nc.dram_tensor("outst", [128, 8], f32,
                                kind="ExternalOutput"),
    }
    with tile.TileContext(nc) as tc:
        _emit(tc, nc, io)
    nc.compile()
    _CACHE[key] = nc
    return nc


def _bf16(a):
    return np.asarray(a, np.float32).astype(ml_dtypes.bfloat16)


def _host_prep(edge_index, Ws, gs, bes):
    """Build the device-layout arrays on host."""
    src = np.asarray(edge_index[0], dtype=np.int64)
    dst = np.asarray(edge_index[1], dtype=np.int64)
    deg = np.zeros(L, np.float32)
    np.add.at(deg, dst, np.float32(1.0))
    deg += np.float32(2.0)
    dis = (1.0 / np.sqrt(deg.astype(np.float64))).astype(np.float32)
    A = np.zeros((L, L), np.float32)
    np.add.at(A, (src, dst), dis[src] * dis[dst])
    A[np.arange(L), np.arange(L)] += np.float32(2.0) * dis * dis
    ash = _bf16(np.ascontiguousarray(
        A.reshape(LT, 128, DQQ, DW).transpose(1, 2, 0, 3).reshape(128, DQQ, LT * DW)))

    wm = np.zeros((6, 128, 512), np.float32)
    for i, W in enumerate(Ws):
        cin = W.shape[0]
        wm[i, :, : (cin // 128) * 256] = np.ascontiguousarray(
            W.reshape(cin // 128, 128, 256).transpose(1, 0, 2).reshape(128, -1))
    wm = _bf16(wm)

    gb = np.zeros((128, 24), np.float32)
    vecs = [gs[0], bes[0], gs[1], bes[1], gs[2], bes[2],
            gs[3], bes[3], gs[4], bes[4], gs[5], bes[5]]
    for v, w in enumerate(vecs):
        gb[:, v * 2 + 0] = w[0:128]
        gb[:, v * 2 + 1] = w[128:256]
    return ash, wm, gb


def kernel(x, edge_index, W1, b1, W2, b2, W3, b3, W1s, b1s, W2s, b2s, W3s, b3s,
           g1, be1, g2, be2, g3, be3, g1s, be1s, g2s, be2s, g3s, be3s):
    x = np.asarray(x, np.float32)
    ash, wm, gb = _host_prep(
        np.asarray(edge_index),
        [np.asarray(W1, np.float32), np.asarray(W2, np.float32),
         np.asarray(W3, np.float32), np.asarray(W1s, np.float32),
         np.asarray(W2s, np.float32), np.asarray(W3s, np.float32)],
        [np.asarray(g1, np.float32), np.asarray(g2, np.float32),
         np.asarray(g3, np.float32), np.asarray(g1s, np.float32),
         np.asarray(g2s, np.float32), np.asarray(g3s, np.float32)],
        [np.asarray(be1, np.float32), np.asarray(be2, np.float32),
         np.asarray(be3, np.float32), np.asarray(be1s, np.float32),
         np.asarray(be2s, np.float32), np.asarray(be3s, np.float32)])

    # core k: graph b=k//2, copies n in [4*(k%2), 4*(k%2)+4)
    # upload x node-major: slot[p, st*128 + c] = x[st*128+p, c]
    xr = x.reshape(NCORES, GPC, CH[0], L)
    xnm = _bf16(np.ascontiguousarray(
        xr.reshape(NCORES, GPC, 128, LT, 128).transpose(0, 1, 4, 3, 2)
          .reshape(NCORES, GPC, 128, LT * 128)))
    # pooled-branch input: max over the 8 copies of each graph, node-major
    xp = x.reshape(B, N, CH[0], L).max(axis=1)  # [B, 128, L]
    xpnm = _bf16(np.ascontiguousarray(
        xp.reshape(B, 128, LT, 128).transpose(0, 3, 2, 1)
          .reshape(B, 128, LT * 128)))
    in_maps = []
    for k in range(NCORES):
        par = k % 2
        in_maps.append({
            "xsh": xnm[k], "pxsh": xpnm[k // 2],
            "Ash": ash, "AshP": np.ascontiguousarray(ash[:, 2 * par:2 * par + 2, :]),
            "Wmats": wm, "gbs": gb,
        })

    nc = _build()

    if _SIMULATE:
        from concourse.bass_interp import MultiCoreSim
        sim = MultiCoreSim(nc, NCORES)
        for k in range(NCORES):
            for nm, arr in in_maps[k].items():
                sim.cores[k].tensor(nm)[:] = arr
        sim.simulate(check_with_hw=False)
        outs = [np.array(sim.cores[k].mem_tensor("out")) for k in range(NCORES)]
        outs2 = [np.array(sim.cores[k].mem_tensor("out2")) for k in range(NCORES)]
        outsst = [np.array(sim.cores[k].mem_tensor("outst")) for k in range(NCORES)]
    else:
        res = run_bass_kernel_spmd(nc, in_maps, core_ids=list(range(NCORES)),
                                   trace=_PROFILE)
        if _PROFILE:
            _CACHE["last_result"] = res
        outs = [np.asarray(res.results[k]["out"]) for k in range(NCORES)]
        outs2 = [np.asarray(res.results[k]["out2"]) for k in range(NCORES)]
        outsst = [np.asarray(res.results[k]["outst"]) for k in range(NCORES)]

    # ---- host-side final layer: BN affine + pooled add + relu ----------
    # buf [GPC, 128, 2*L] bf16: y[g, cot*128+p, n] = buf[g, p, cot*L+n]
    y1 = np.stack([o.astype(np.float32) for o in outs])        # [8,G,128,2L]
    y1 = (y1.reshape(NCORES, GPC, 128, 2, L).transpose(0, 1, 3, 2, 4)
            .reshape(NCORES, GPC, 256, L))
    # out2 halves: core 2b+par holds dest-node cols (2*par+d2)*512+j
    y2 = np.empty((B, 256, L), np.float32)
    for b in range(B):
        for par in range(2):
            h = outs2[2 * b + par].astype(np.float32)  # [128, 4*DW]
            h = h.reshape(128, 2, 2, DW)               # [p, cot, d2, j]
            for cot in range(2):
                for d2 in range(2):
                    nd = (2 * par + d2) * DW
                    y2[b, cot * 128:(cot + 1) * 128, nd:nd + DW] = \
                        h[:, cot, d2, :]
    st = np.sum(np.stack([o.astype(np.float64) for o in outsst]), axis=0)

    def bn_affine(s1, s2, cnt, g, be):
        m = s1 / cnt
        v = s2 / cnt - m * m
        a = np.asarray(g, np.float64) / np.sqrt(v + EPS)
        return (a.astype(np.float32),
                (np.asarray(be, np.float64) - a * m).astype(np.float32))

    a1, b1 = bn_affine(st[:, 0:2].T.reshape(256), st[:, 2:4].T.reshape(256),
                       CNT_E, g3, be3)
    a2, b2 = bn_affine(st[:, 4:6].T.reshape(256), st[:, 6:8].T.reshape(256),
                       CNT_P3, g3s, be3s)
    bsum = (b1 + b2)[None, :, None]
    out = np.empty((NCORES * GPC, 256, L), np.float32)
    for k in range(NCORES):
        out[k * GPC:(k + 1) * GPC] = (a1[None, :, None] * y1[k]
                                      + a2[None, :, None] * y2[k // 2] + bsum)
    np.maximum(out, 0.0, out=out)
    return out
